# revision 1
# baseline (speedup 1.0000x reference)
"""Trainium2 Bass kernel for a dense transformer block (pre-LN attention + GELU MLP).

Strategy: data-parallel over batch across 8 NeuronCores (2 batches/core, no
collectives).  Per core: token-major residual stream with feature-major
activations for matmuls (PE-transpose at the two LayerNorms), fp32r matmuls
(full PE rate), softmax without max-subtraction (scores are O(1) bounded by
construction), PV matmul with a ones-column on V to produce row-sums for free.
"""

import numpy as np

import concourse.bass as bass
import concourse.mybir as mybir
import concourse.tile as tile
from concourse import bacc, bass_utils
from concourse.masks import make_identity

# Problem shape (hardcoded per spec nn_Block_58652073394865)
B, S, D, H, F = 16, 577, 1024, 16, 4096
DH = D // H
NCORES = 8
BL = B // NCORES        # batches per core
P = 128
KK = D // P             # 8 chunks of the model dim
FK = F // P             # 32 chunks of the mlp dim
EPS = 1e-6

# fp32r matmuls require even free-dim counts, so pad tokens 577 -> 578 (one
# zeroed pad token) and use even, overlapping moving-token chunks.
SP = 578
TT = [(0, 128), (128, 128), (256, 128), (384, 128), (512, 66)]   # token tiles (incl pad)
QC = [(0, 290), (288, 290)]                                      # moving-token chunks (even, >=256)
DC = [(0, 512), (512, 512)]                                      # model-dim 512 chunks
VS = 66                                                          # per-head stride in v (64 v + 1 ones + 1 pad)

F32 = mybir.dt.float32
F32R = mybir.dt.float32r
AF = mybir.ActivationFunctionType
OP = mybir.AluOpType

WEIGHT_NAMES = [
    "ln1_g", "ln1_b", "wq", "bq", "wk", "bk", "wv", "bv", "wo", "bo",
    "ln2_g", "ln2_b", "w1", "b1", "w2", "b2",
]

_NC_CACHE = None
# CoreSim doesn't implement the Gelu LUT; tests may swap this for AF.Tanh
_GELU = AF.Gelu


def _build():
    nc = bacc.Bacc("TRN2", target_bir_lowering=False, debug=False,
                   num_devices=NCORES)

    x_d = nc.dram_tensor("x", [BL, S, D], F32, kind="ExternalInput").ap()
    y_d = nc.dram_tensor("y", [BL, S, D], F32, kind="ExternalOutput").ap()
    # weights consumed by matmuls -> declare fp32r (same bits as fp32)
    wq_d = nc.dram_tensor("wq", [D, D], F32R, kind="ExternalInput").ap()
    wk_d = nc.dram_tensor("wk", [D, D], F32R, kind="ExternalInput").ap()
    wv_d = nc.dram_tensor("wv", [D, D], F32R, kind="ExternalInput").ap()
    wo_d = nc.dram_tensor("wo", [D, D], F32R, kind="ExternalInput").ap()
    w1_d = nc.dram_tensor("w1", [D, F], F32R, kind="ExternalInput").ap()
    w2_d = nc.dram_tensor("w2", [F, D], F32R, kind="ExternalInput").ap()
    bv_d = nc.dram_tensor("bv", [D], F32R, kind="ExternalInput").ap()   # folded via K=1 matmul
    bo_d = nc.dram_tensor("bo", [D], F32R, kind="ExternalInput").ap()   # folded via K=1 matmul
    bq_d = nc.dram_tensor("bq", [D], F32, kind="ExternalInput").ap()
    bk_d = nc.dram_tensor("bk", [D], F32, kind="ExternalInput").ap()
    b1_d = nc.dram_tensor("b1", [F], F32, kind="ExternalInput").ap()
    b2_d = nc.dram_tensor("b2", [D], F32, kind="ExternalInput").ap()
    g1_d = nc.dram_tensor("ln1_g", [D], F32, kind="ExternalInput").ap()
    gb1_d = nc.dram_tensor("ln1_b", [D], F32, kind="ExternalInput").ap()
    g2_d = nc.dram_tensor("ln2_g", [D], F32, kind="ExternalInput").ap()
    gb2_d = nc.dram_tensor("ln2_b", [D], F32, kind="ExternalInput").ap()

    wq_r = wq_d.rearrange("(ko p) d -> p ko d", p=P)
    wk_r = wk_d.rearrange("(ko p) d -> p ko d", p=P)
    wv_r = wv_d.rearrange("(ko p) d -> p ko d", p=P)
    wo_r = wo_d.rearrange("(ko p) d -> p ko d", p=P)
    w1_r = w1_d.rearrange("(ko p) d -> p ko d", p=P)
    w2_r = w2_d.rearrange("(ko p) d -> p ko d", p=P)

    with tile.TileContext(nc) as tc:
        with tc.tile_pool(name="const", bufs=1) as cpool, \
             tc.tile_pool(name="resid", bufs=2) as rpool, \
             tc.tile_pool(name="fmbuf", bufs=1) as fmpool, \
             tc.tile_pool(name="ostg", bufs=4) as opool, \
             tc.tile_pool(name="lnp", bufs=2) as lnpool, \
             tc.tile_pool(name="psA", bufs=4, space="PSUM") as psA:

            # ---- constants / small params ----
            # tiles pad to 4KB/partition: pack the small params into few tiles
            cA = cpool.tile([P, 7 * KK + FK], F32, tag="cA")
            bq_sb = cA[:, 0:KK]
            bk_sb = cA[:, KK:2 * KK]
            b2_sb = cA[:, 2 * KK:3 * KK]
            g1_sb = cA[:, 3 * KK:4 * KK]
            gb1_sb = cA[:, 4 * KK:5 * KK]
            g2_sb = cA[:, 5 * KK:6 * KK]
            gb2_sb = cA[:, 6 * KK:7 * KK]
            b1_sb = cA[:, 7 * KK:7 * KK + FK]
            nc.sync.dma_start(bq_sb, bq_d.rearrange("(m p) -> p m", p=P))
            nc.sync.dma_start(bk_sb, bk_d.rearrange("(m p) -> p m", p=P))
            nc.sync.dma_start(b2_sb, b2_d.rearrange("(m p) -> p m", p=P))
            nc.sync.dma_start(g1_sb, g1_d.rearrange("(c p) -> p c", p=P))
            nc.sync.dma_start(gb1_sb, gb1_d.rearrange("(c p) -> p c", p=P))
            nc.sync.dma_start(g2_sb, g2_d.rearrange("(c p) -> p c", p=P))
            nc.sync.dma_start(gb2_sb, gb2_d.rearrange("(c p) -> p c", p=P))
            nc.sync.dma_start(b1_sb, b1_d.rearrange("(m p) -> p m", p=P))

            cB = cpool.tile([P, P + 2], F32, tag="cB")
            ident = cB[:, 0:P]
            epsap = cB[:, P:P + 1]
            onec_f = cB[:, P + 1:P + 2]
            make_identity(nc, ident)
            nc.vector.memset(epsap, EPS)
            nc.vector.memset(onec_f, 1.0)

            ident_r = cpool.tile([P, P], F32R, tag="ident_r")
            nc.vector.tensor_copy(ident_r[:], ident)

            ones_f = cpool.tile([1, P], F32, tag="ones_f")
            nc.vector.memset(ones_f[:], 1.0)
            cD = cpool.tile([1, P + 2 * D], F32R, tag="cD")
            ones_r = cD[:, 0:P]
            t_bo = cD[:, P:P + D]
            t_bv = cD[:, P + D:P + 2 * D]
            nc.vector.tensor_copy(ones_r, ones_f[:])
            nc.sync.dma_start(t_bo, bo_d[None, :])
            nc.sync.dma_start(t_bv, bv_d[None, :])

            # token-major layernorm -> feature-major normalized output
            def ln_new_stats(ln_pool):
                stats = ln_pool.tile([P, 20], F32, tag="stats")
                # last token tile covers only 66 partitions; keep the rest defined
                nc.vector.memset(stats[:, 0:5], 0.0)
                nc.vector.memset(stats[:, 5:10], 1.0)
                return stats

            def ln_tile_stats(ln_pool, stats, src, ti, pt):
                negmu = stats[:, 0:5]
                varD = stats[:, 5:10]
                nc.vector.tensor_reduce(
                    negmu[:pt, ti:ti + 1], src[:pt, ti],
                    mybir.AxisListType.X, OP.add)
                nc.vector.tensor_scalar_mul(
                    negmu[:pt, ti:ti + 1], negmu[:pt, ti:ti + 1], -1.0 / D)
                scr = ln_pool.tile([P, D], F32R, tag="xn_tm", bufs=3)
                nc.scalar.activation(
                    scr[:pt], src[:pt, ti], AF.Square,
                    bias=negmu[:pt, ti:ti + 1], accum_out=varD[:pt, ti:ti + 1])

            def ln_finalize(stats, lo, hi):
                # rsig for tile range [lo, hi)
                nc.scalar.activation(stats[:, 10 + lo:10 + hi],
                                     stats[:, 5 + lo:5 + hi], AF.Sqrt,
                                     scale=1.0 / D, bias=epsap[:])
                nc.vector.reciprocal(stats[:, 15 + lo:15 + hi],
                                     stats[:, 10 + lo:10 + hi])

            def ln_apply_tiles(ln_pool, stats, src, g_sb, gb_sb, dst_fm, tis):
                negmu = stats[:, 0:5]
                rsig = stats[:, 15:20]
                for ti in tis:
                    t0, pt = TT[ti]
                    xn = ln_pool.tile([P, D], F32R, tag="xn_tm", bufs=3)
                    nc.vector.tensor_scalar(
                        xn[:pt], src[:pt, ti],
                        negmu[:pt, ti:ti + 1], rsig[:pt, ti:ti + 1],
                        OP.add, OP.mult)
                    for kk in range(KK):
                        pst = psA.tile([P, 512], F32R, tag="pA")
                        nc.tensor.transpose(
                            pst[:, :pt], xn[:pt, kk * P:(kk + 1) * P],
                            ident_r[:pt, :pt])
                        nc.vector.scalar_tensor_tensor(
                            dst_fm[:, kk, t0:t0 + pt], pst[:, :pt],
                            g_sb[:, kk:kk + 1],
                            gb_sb[:, kk:kk + 1].to_broadcast((P, pt)),
                            OP.mult, OP.add)

            def layer_norm_fm(ln_pool, src, g_sb, gb_sb, dst_fm):
                stats = ln_new_stats(ln_pool)
                for ti, (t0, pt) in enumerate(TT):
                    ln_tile_stats(ln_pool, stats, src, ti, pt)
                # finalize tile 0 alone so its transposes start after one x-tile
                ln_finalize(stats, 0, 1)
                ln_apply_tiles(ln_pool, stats, src, g_sb, gb_sb, dst_fm, (0,))
                ln_finalize(stats, 1, 4)
                ln_apply_tiles(ln_pool, stats, src, g_sb, gb_sb, dst_fm, (1, 2, 3))
                ln_finalize(stats, 4, 5)
                ln_apply_tiles(ln_pool, stats, src, g_sb, gb_sb, dst_fm, (4,))

            for b in range(BL):
                xn_fm = fmpool.tile([P, KK, SP], F32R, tag="xn_fm")
                xb = rpool.tile([P, 5, D], F32, tag="resid")

                # ---- stage A: load x (token-major); zero the pad token row ----
                # (engine start-partition must be a multiple of 32: zero 64..127
                # first, then the DMA rewrites the real rows 0..64)
                nc.vector.memset(xb[64:, 4, :], 0.0)
                for ti, (t0, pt) in enumerate(TT):
                    rp = min(pt, S - t0)   # real (non-pad) tokens in this tile
                    nc.sync.dma_start(xb[:rp, ti], x_d[b, t0:t0 + rp, :])

                # ---- stage B: LN1 -> xn_fm ----
                layer_norm_fm(lnpool, xb, g1_sb, gb1_sb, xn_fm)

                with tc.tile_pool(name="attn", bufs=1) as apool, \
                     tc.tile_pool(name="wblk", bufs=2) as wpool:
                    q_fm = apool.tile([P, KK, SP], F32R, tag="q")
                    k_fm = apool.tile([P, KK, SP], F32R, tag="k")
                    v_sb = apool.tile([P, 5, H * VS], F32R, tag="v")
                    ctx_fm = apool.tile([P, KK, SP], F32R, tag="ctx")

                    # col 64 of each head's stride-66 group = 1 (rowsum trick),
                    # col 65 = 0 (fp32r even-M pad).  The pad token's whole v
                    # row (tile 4, partition 65) must be zero: zero partitions
                    # 64.. first, later writes refill only the real rows.
                    v_hc = v_sb[:].rearrange("p t (h c) -> p t h c", c=VS)
                    # memset can't target fp32r; zero via a uint32 view
                    nc.vector.memset(v_hc[64:, 4:5].bitcast(mybir.dt.uint32), 0)
                    nc.vector.memset(v_hc[:, :, :, 65:66].bitcast(mybir.dt.uint32), 0)
                    nc.vector.tensor_copy(
                        v_hc[:, 0:4, :, 64:65],
                        onec_f[:, :, None, None].to_broadcast((P, 4, H, 1)))
                    nc.vector.tensor_copy(
                        v_hc[:65, 4:5, :, 64:65],
                        onec_f[:65, :, None, None].to_broadcast((65, 1, H, 1)))

                    # ---- stage C/D interleaved: projections + attention ----
                    # blk covers q/k m-tiles 4*blk..4*blk+3 and v heads
                    # 8*blk..8*blk+7 == attention heads 8*blk..8*blk+7, so each
                    # half's projections feed its attention while the NEXT
                    # half's projection matmuls fill the exp-bound PE idle.
                    def emit_qk(blk):
                        for w_r, bias_sb, dst in ((wq_r, bq_sb, q_fm), (wk_r, bk_sb, k_fm)):
                            wb = wpool.tile([P, KK, 512], F32R, tag="wblk")
                            nc.sync.dma_start(wb[:], w_r[:, :, blk * 512:(blk + 1) * 512])
                            for mi in range(4):
                                m = blk * 4 + mi
                                for (q0, qn) in QC:
                                    ps = psA.tile([P, 512], F32, tag="pA")
                                    for kk in range(KK):
                                        nc.tensor.matmul(
                                            ps[:, :qn],
                                            wb[:, kk, mi * P:(mi + 1) * P],
                                            xn_fm[:, kk, q0:q0 + qn],
                                            start=(kk == 0), stop=(kk == KK - 1))
                                    nc.scalar.activation(
                                        dst[:, m, q0:q0 + qn], ps[:, :qn],
                                        AF.Identity, bias=bias_sb[:, m:m + 1])

                    def emit_v(ci):
                        c0, cn = DC[ci]
                        wb = wpool.tile([P, KK, 512], F32R, tag="wblk")
                        nc.sync.dma_start(wb[:], wv_r[:, :, c0:c0 + cn])
                        for ti, (t0, pt) in enumerate(TT):
                            ps = psA.tile([P, 512], F32, tag="pA")
                            for kk in range(KK):
                                nc.tensor.matmul(
                                    ps[:pt], xn_fm[:, kk, t0:t0 + pt],
                                    wb[:, kk, :], start=(kk == 0), stop=False)
                            nc.tensor.matmul(
                                ps[:pt], ones_r[:, :pt], t_bv[:, c0:c0 + cn],
                                start=False, stop=True)
                            rp = min(pt, S - t0)
                            nc.vector.tensor_copy(
                                v_sb[:rp, ti].rearrange("p (h c) -> p h c", c=VS)[:, ci * 8:(ci + 1) * 8, 0:64],
                                ps[:rp, :cn].rearrange("p (h c) -> p h c", c=64))

                    def emit_attn(h):
                        hrow = (h % 2) * 64
                        kkh = h // 2
                        for qi, (q0, qn) in enumerate(QC):
                            es = apool.tile([P, 5, qn], F32R, tag=f"es{qi}")
                            # pair the 5 score tiles into 2-bank psum groups so
                            # each Exp covers 2 tiles (halves the per-op cost)
                            for pair in ((0, 1), (2, 3), (4,)):
                                pg = psA.tile([P, 2, 512], F32, tag="pS", bufs=2)
                                for j, kt in enumerate(pair):
                                    t0, ptk = TT[kt]
                                    nc.tensor.matmul(
                                        pg[:ptk, j, :qn],
                                        k_fm[hrow:hrow + 64, kkh, t0:t0 + ptk],
                                        q_fm[hrow:hrow + 64, kkh, q0:q0 + qn],
                                        start=True, stop=True)
                                npair = len(pair)
                                prow = TT[pair[0]][1]   # 128 for full pairs, 66 for (4,)
                                nc.scalar.activation(
                                    es[:prow, pair[0]:pair[0] + npair, :],
                                    pg[:prow, :npair, :qn],
                                    AF.Exp, scale=1.0 / np.sqrt(DH))
                            pc = psA.tile([VS, 512], F32, tag="pA")
                            for kt, (t0, ptk) in enumerate(TT):
                                nc.tensor.matmul(
                                    pc[:, :qn],
                                    v_sb[:ptk, kt, h * VS:(h + 1) * VS],
                                    es[:ptk, kt, :],
                                    start=(kt == 0), stop=(kt == 4))
                            rc = apool.tile([1, 290], F32, tag="rc", bufs=2)
                            nc.vector.reciprocal(rc[:, :qn], pc[64:65, :qn])
                            rb = apool.tile([64, 290], F32, tag="rb", bufs=2)
                            nc.gpsimd.partition_broadcast(rb[:, :qn], rc[:, :qn])
                            nc.vector.tensor_tensor(
                                ctx_fm[hrow:hrow + 64, kkh, q0:q0 + qn],
                                pc[0:64, :qn], rb[:, :qn], OP.mult)

                    emit_qk(0)
                    emit_v(0)
                    for h in range(8):
                        emit_attn(h)
                    emit_qk(1)
                    emit_v(1)
                    for h in range(8, H):
                        emit_attn(h)

                    # ---- stage E: output projection + residual -> x2,
                    # with LN2 folded in per-tile ----
                    x2 = rpool.tile([P, 5, D], F32, tag="resid")
                    xn2_fm = fmpool.tile([P, KK, SP], F32R, tag="xn_fm")
                    stats2 = ln_new_stats(lnpool)
                    for ci, (c0, cn) in enumerate(DC):
                        wb = wpool.tile([P, KK, 512], F32R, tag="wblk")
                        nc.sync.dma_start(wb[:], wo_r[:, :, c0:c0 + cn])
                        for ti, (t0, pt) in enumerate(TT):
                            ps = psA.tile([P, 512], F32, tag="pA")
                            for kk in range(KK):
                                nc.tensor.matmul(
                                    ps[:pt], ctx_fm[:, kk, t0:t0 + pt],
                                    wb[:, kk, :], start=(kk == 0), stop=False)
                            nc.tensor.matmul(
                                ps[:pt], ones_r[:, :pt], t_bo[:, c0:c0 + cn],
                                start=False, stop=True)
                            nc.vector.scalar_tensor_tensor(
                                x2[:pt, ti, c0:c0 + cn], ps[:pt], 0.0,
                                xb[:pt, ti, c0:c0 + cn], OP.add, OP.add)
                            if ci == len(DC) - 1:
                                # x2 tile complete: fold its LN2 stats in now
                                ln_tile_stats(lnpool, stats2, x2, ti, pt)



                # ---- stage F: LN2 apply ----
                ln_finalize(stats2, 0, 4)
                ln_apply_tiles(lnpool, stats2, x2, g2_sb, gb2_sb, xn2_fm, (0, 1, 2, 3))
                ln_finalize(stats2, 4, 5)
                ln_apply_tiles(lnpool, stats2, x2, g2_sb, gb2_sb, xn2_fm, (4,))

                # ---- stage G: MLP ----
                with tc.tile_pool(name="mlp", bufs=1) as mpool, \
                     tc.tile_pool(name="wmlp", bufs=2) as mwpool:
                    h1 = mpool.tile([P, FK, SP], F32R, tag="h1")
                    _psc = [0]

                    def mlp_psum():
                        # pS's 2x2 banks are idle during MLP: every 3rd group
                        # borrows one -> 6 accumulation groups in flight
                        _psc[0] += 1
                        if _psc[0] % 3 == 0:
                            t = psA.tile([P, 2, 512], F32, tag="pS", bufs=2,
                                         name="ps_alt")
                            return t[:, 0]
                        return psA.tile([P, 512], F32, tag="pA", name="ps_a")

                    for blk in range(8):
                        wb = mwpool.tile([P, KK, 512], F32R, tag="wmlp")
                        nc.sync.dma_start(wb[:], w1_r[:, :, blk * 512:(blk + 1) * 512])
                        for mi in range(4):
                            m = blk * 4 + mi
                            for (q0, qn) in QC:
                                ps = mlp_psum()
                                for kk in range(KK):
                                    nc.tensor.matmul(
                                        ps[:, :qn],
                                        wb[:, kk, mi * P:(mi + 1) * P],
                                        xn2_fm[:, kk, q0:q0 + qn],
                                        start=(kk == 0), stop=(kk == KK - 1))
                                nc.scalar.activation(
                                    h1[:, m, q0:q0 + qn], ps[:, :qn],
                                    _GELU, bias=b1_sb[:, m:m + 1])
                    mlp_fm = mpool.tile([P, KK, SP], F32R, tag="mlp_fm")
                    for m in range(KK):
                        wb = mwpool.tile([P, FK, P], F32R, tag="wmlp")
                        nc.sync.dma_start(wb[:], w2_r[:, :, m * P:(m + 1) * P])
                        for (q0, qn) in QC:
                            ps = mlp_psum()
                            for kk2 in range(FK):
                                nc.tensor.matmul(
                                    ps[:, :qn], wb[:, kk2],
                                    h1[:, kk2, q0:q0 + qn],
                                    start=(kk2 == 0), stop=(kk2 == FK - 1))
                            nc.vector.tensor_scalar_add(
                                mlp_fm[:, m, q0:q0 + qn], ps[:, :qn],
                                b2_sb[:, m:m + 1])
                        # this m's feature rows are complete: transpose back to
                        # token-major, add residual, store (interleaves with the
                        # next m's w2 matmuls)
                        for ti, (t0, pt) in enumerate(TT):
                            rp = min(pt, S - t0)   # skip the pad token on store
                            ps = psA.tile([P, 512], F32R, tag="pA")
                            nc.tensor.transpose(
                                ps[:pt, :P], mlp_fm[:, m, t0:t0 + pt], ident_r[:])
                            og = opool.tile([P, P], F32, tag="ostg", bufs=6)
                            nc.vector.scalar_tensor_tensor(
                                og[:pt], ps[:pt, :P], 0.0,
                                x2[:pt, ti, m * P:(m + 1) * P], OP.add, OP.add)
                            nc.sync.dma_start(
                                y_d[b, t0:t0 + rp, m * P:(m + 1) * P], og[:rp])

    nc.compile()
    return nc


def _get_nc():
    global _NC_CACHE
    if _NC_CACHE is None:
        _NC_CACHE = _build()
    return _NC_CACHE


def kernel(**inputs):
    nc = _get_nc()
    x = np.ascontiguousarray(np.asarray(inputs["x"], dtype=np.float32))
    shared = {
        n: np.ascontiguousarray(np.asarray(inputs[n], dtype=np.float32))
        for n in WEIGHT_NAMES
    }
    in_maps = []
    for i in range(NCORES):
        m = dict(shared)
        m["x"] = np.ascontiguousarray(x[i * BL:(i + 1) * BL])
        in_maps.append(m)
    res = bass_utils.run_bass_kernel_spmd(nc, in_maps, core_ids=list(range(NCORES)))
    y = np.concatenate([res.results[i]["y"] for i in range(NCORES)], axis=0)
    return y.astype(np.float32)



# revision 7
# speedup vs baseline: 1.1538x; 1.1538x over previous
"""Trainium2 Bass kernel for a dense transformer block (pre-LN attention +
GELU MLP) — fp8e4m3 DoubleRow edition.

Strategy: data-parallel over batch across 8 NeuronCores (2 batches/core, no
collectives).  All matmuls run in fp8e4m3 with MatmulPerfMode.DoubleRow
(2 k-tiles per instruction at 0.5 cycles/row = 4x the fp32r rate).  Accuracy
is held by:
  - weights pre-scaled by 64/128 into fp8's normal range (inverse scale is
    folded into the free scalar slots of psum->SBUF copy ops),
  - residual-split operands: the MLP input and hidden activations are
    represented as main+residual fp8 pairs (a1+a2, h1+h2), and w1/w2 carry a
    matched-scale fp8 residual term (w1b/w2b), so the MLP is computed to
    ~0.3% while still running entirely at DoubleRow rate,
  - softmax without max-subtraction: es = exp(s - 3.2) stored in fp8 (the
    constant bias cancels in the normalization; max score ~8.2 so es < 240).
LayerNorm beta terms are folded into the following layer's biases on the host
(exactly linear), so the on-chip LN applies only the gain.
"""

import numpy as np
import ml_dtypes

import concourse.bass as bass
import concourse.mybir as mybir
import concourse.tile as tile
from concourse import bacc, bass_utils
from concourse.masks import make_identity

# Problem shape (hardcoded per spec nn_Block_58652073394865)
B, S, D, H, F = 16, 577, 1024, 16, 4096
DH = D // H
NCORES = 8
BL = B // NCORES        # batches per core
P = 128
KK = D // P             # 8 chunks of the model dim
FK = F // P             # 32 chunks of the mlp dim
EPS = 1e-6

SP = 578                # tokens padded with one zero token
TT = [(0, 128), (128, 128), (256, 128), (384, 128), (512, 66)]
NC = [(0, 256), (256, 256), (512, 66)]   # DoubleRow moving chunks (out <= 256)
SC = [(0, 512), (512, 66)]               # plain-fp8 score chunks (out <= 512)
VS = 66                 # per-head stride in v (64 v + 1 ones + 1 pad)

WSC = 64.0              # fp8 pre-scale for wq/wk/wv/wo/w1
W2SC = 128.0            # fp8 pre-scale for w2
CTXSC = 16.0            # fp8 pre-scale for ctx
EXPB = 3.2              # softmax exp bias (cancels in normalization)
FC1_TERMS = 3           # 2: a1@w1a + a2@w1a;  3: + a1@w1b
FC2_TERMS = 3           # 2: h1@w2a + h2@w2a;  3: + h1@w2b

F32 = mybir.dt.float32
BF16 = mybir.dt.bfloat16
FP8 = mybir.dt.float8e4
U32 = mybir.dt.uint32
AF = mybir.ActivationFunctionType
OP = mybir.AluOpType
DR = mybir.MatmulPerfMode.DoubleRow

E4NP = ml_dtypes.float8_e4m3
BFNP = ml_dtypes.bfloat16

_NC_CACHE = None
# CoreSim doesn't implement the Gelu LUT; tests may swap this for AF.Tanh
_GELU = AF.Gelu

SHARED_NAMES = ["wq", "wk", "wv", "wo", "w1a", "w2a", "bq", "bk", "bv", "bo",
                "b1", "b2", "g1", "g2"]
if FC1_TERMS == 3:
    SHARED_NAMES.append("w1b")
if FC2_TERMS == 3:
    SHARED_NAMES.append("w2b")


def prepare_shared(inputs):
    """Host-side: quantize/scale weights, fold LN betas into biases."""
    f = {n: np.ascontiguousarray(np.asarray(inputs[n], np.float32))
         for n in ("wq", "wk", "wv", "wo", "w1", "w2", "bq", "bk", "bv", "bo",
                   "b1", "b2", "ln1_g", "ln1_b", "ln2_g", "ln2_b")}

    def q8s(w, s):
        return np.ascontiguousarray((w * s).astype(E4NP))

    out = {
        "wq": q8s(f["wq"], WSC), "wk": q8s(f["wk"], WSC),
        "wv": q8s(f["wv"], WSC), "wo": q8s(f["wo"], WSC),
        "g1": f["ln1_g"], "g2": f["ln2_g"],
        "bq": f["bq"] + f["ln1_b"] @ f["wq"],
        "bk": f["bk"] + f["ln1_b"] @ f["wk"],
        "bv": f["bv"] + f["ln1_b"] @ f["wv"],
        "bo": np.ascontiguousarray(
            (f["bo"] * (WSC * CTXSC)).astype(BFNP)),
        "b1": f["b1"] + f["ln2_b"] @ f["w1"],
        "b2": f["b2"],
    }
    w1s = f["w1"] * WSC
    w1a = w1s.astype(E4NP)
    out["w1a"] = np.ascontiguousarray(w1a)
    if FC1_TERMS == 3:
        out["w1b"] = np.ascontiguousarray(
            (w1s - w1a.astype(np.float32)).astype(E4NP))
    w2s = f["w2"] * W2SC
    w2a = w2s.astype(E4NP)
    out["w2a"] = np.ascontiguousarray(w2a)
    if FC2_TERMS == 3:
        out["w2b"] = np.ascontiguousarray(
            (w2s - w2a.astype(np.float32)).astype(E4NP))
    return out


def _build():
    nc = bacc.Bacc("TRN2", target_bir_lowering=False, debug=False,
                   num_devices=NCORES)

    x_d = nc.dram_tensor("x", [BL, S, D], BF16, kind="ExternalInput").ap()
    y_d = nc.dram_tensor("y", [BL, S, D], BF16, kind="ExternalOutput").ap()
    wq_d = nc.dram_tensor("wq", [D, D], FP8, kind="ExternalInput").ap()
    wk_d = nc.dram_tensor("wk", [D, D], FP8, kind="ExternalInput").ap()
    wv_d = nc.dram_tensor("wv", [D, D], FP8, kind="ExternalInput").ap()
    wo_d = nc.dram_tensor("wo", [D, D], FP8, kind="ExternalInput").ap()
    w1a_d = nc.dram_tensor("w1a", [D, F], FP8, kind="ExternalInput").ap()
    w2a_d = nc.dram_tensor("w2a", [F, D], FP8, kind="ExternalInput").ap()
    w1b_d = (nc.dram_tensor("w1b", [D, F], FP8, kind="ExternalInput").ap()
             if FC1_TERMS == 3 else None)
    w2b_d = (nc.dram_tensor("w2b", [F, D], FP8, kind="ExternalInput").ap()
             if FC2_TERMS == 3 else None)
    bq_d = nc.dram_tensor("bq", [D], F32, kind="ExternalInput").ap()
    bk_d = nc.dram_tensor("bk", [D], F32, kind="ExternalInput").ap()
    bv_d = nc.dram_tensor("bv", [D], F32, kind="ExternalInput").ap()
    bo_d = nc.dram_tensor("bo", [D], BF16, kind="ExternalInput").ap()
    b1_d = nc.dram_tensor("b1", [F], F32, kind="ExternalInput").ap()
    b2_d = nc.dram_tensor("b2", [D], F32, kind="ExternalInput").ap()
    g1_d = nc.dram_tensor("g1", [D], F32, kind="ExternalInput").ap()
    g2_d = nc.dram_tensor("g2", [D], F32, kind="ExternalInput").ap()

    wq_r = wq_d.rearrange("(ko p) d -> p ko d", p=P)
    wk_r = wk_d.rearrange("(ko p) d -> p ko d", p=P)
    wv_r = wv_d.rearrange("(ko p) d -> p ko d", p=P)
    wo_r = wo_d.rearrange("(ko p) d -> p ko d", p=P)
    w1a_r = w1a_d.rearrange("(ko p) d -> p ko d", p=P)
    w2a_r = w2a_d.rearrange("(ko p) d -> p ko d", p=P)
    w1b_r = w1b_d.rearrange("(ko p) d -> p ko d", p=P) if w1b_d else None
    w2b_r = w2b_d.rearrange("(ko p) d -> p ko d", p=P) if w2b_d else None

    with tile.TileContext(nc) as tc:
        with tc.tile_pool(name="const", bufs=1) as cpool, \
             tc.tile_pool(name="resid", bufs=1) as rpool, \
             tc.tile_pool(name="fmbuf", bufs=1) as fmpool, \
             tc.tile_pool(name="lnp", bufs=2) as lnpool, \
             tc.tile_pool(name="ystg", bufs=1) as ypool, \
             tc.tile_pool(name="psA", bufs=4, space="PSUM") as psA:

            # ---- small params, packed ----
            cA = cpool.tile([P, 5 * KK + FK], F32, tag="cA")
            bq_sb = cA[:, 0:KK]
            bk_sb = cA[:, KK:2 * KK]
            b2_sb = cA[:, 2 * KK:3 * KK]
            g1_sb = cA[:, 3 * KK:4 * KK]
            g2_sb = cA[:, 4 * KK:5 * KK]
            b1_sb = cA[:, 5 * KK:5 * KK + FK]
            nc.sync.dma_start(bq_sb, bq_d.rearrange("(m p) -> p m", p=P))
            nc.sync.dma_start(bk_sb, bk_d.rearrange("(m p) -> p m", p=P))
            nc.sync.dma_start(b2_sb, b2_d.rearrange("(m p) -> p m", p=P))
            nc.sync.dma_start(g1_sb, g1_d.rearrange("(c p) -> p c", p=P))
            nc.sync.dma_start(g2_sb, g2_d.rearrange("(c p) -> p c", p=P))
            nc.sync.dma_start(b1_sb, b1_d.rearrange("(m p) -> p m", p=P))

            cB = cpool.tile([P, P + 3], F32, tag="cB")
            ident_f = cB[:, 0:P]
            epsap = cB[:, P:P + 1]
            onef = cB[:, P + 1:P + 2]
            nexpb = cB[:, P + 2:P + 3]
            make_identity(nc, ident_f)
            nc.vector.memset(epsap, EPS)
            nc.vector.memset(onef, 1.0)
            nc.vector.memset(nexpb, -EXPB)

            cC = cpool.tile([P, P + 4], BF16, tag="cC")
            ident_bf = cC[:, 0:P]
            nc.vector.tensor_copy(ident_bf, ident_f)
            cD = cpool.tile([1, P + D], BF16, tag="cD")
            ones_bf = cD[:, 0:P]
            bo_sb = cD[:, P:P + D]
            nc.vector.memset(ones_bf, 1.0)
            nc.sync.dma_start(bo_sb, bo_d[None, :])

            onec8 = cpool.tile([P, 4], FP8, tag="onec8")
            nc.vector.tensor_copy(onec8[:, 0:1], onef)

            # bv broadcast to all partitions (bias varies along free dim)
            bvrow = cpool.tile([1, D], F32, tag="bvrow")
            nc.sync.dma_start(bvrow[:], bv_d[None, :])
            bvb = cpool.tile([P, D], F32, tag="bvb")
            nc.gpsimd.partition_broadcast(bvb[:], bvrow[:])

            # ---- LayerNorm helpers (token-major stats; gain-only apply) ----
            def ln_new_stats():
                stats = lnpool.tile([P, 20], F32, tag="stats")
                nc.vector.memset(stats[:, 0:5], 0.0)
                nc.vector.memset(stats[:, 5:10], 1.0)
                return stats

            def ln_tile_stats(stats, src, ti, pt):
                negmu = stats[:, 0:5]
                varD = stats[:, 5:10]
                nc.vector.tensor_reduce(
                    negmu[:pt, ti:ti + 1], src[:pt, ti],
                    mybir.AxisListType.X, OP.add)
                nc.vector.tensor_scalar_mul(
                    negmu[:pt, ti:ti + 1], negmu[:pt, ti:ti + 1], -1.0 / D)
                scr = lnpool.tile([P, D], BF16, tag="xsq", bufs=2)
                nc.scalar.activation(
                    scr[:pt], src[:pt, ti], AF.Square,
                    bias=negmu[:pt, ti:ti + 1], accum_out=varD[:pt, ti:ti + 1])

            def ln_finalize(stats, lo, hi):
                nc.scalar.activation(stats[:, 10 + lo:10 + hi],
                                     stats[:, 5 + lo:5 + hi], AF.Sqrt,
                                     scale=1.0 / D, bias=epsap[:])
                nc.vector.reciprocal(stats[:, 15 + lo:15 + hi],
                                     stats[:, 10 + lo:10 + hi])

            def ln_apply_tiles(stats, src, g_sb, dst1, dst2, tis):
                # normalize (bf16), PE-transpose, apply gain -> fp8 fm tiles.
                # dst2 (if not None) receives the fp8 residual split.
                negmu = stats[:, 0:5]
                rsig = stats[:, 15:20]
                for ti in tis:
                    t0, pt = TT[ti]
                    xn = lnpool.tile([P, D], BF16, tag="xn_tm", bufs=2)
                    nc.vector.tensor_scalar(
                        xn[:pt], src[:pt, ti],
                        negmu[:pt, ti:ti + 1], rsig[:pt, ti:ti + 1],
                        OP.add, OP.mult)
                    for kk in range(KK):
                        pst = psA.tile([P, 512], F32, tag="pA")
                        pbf = pst[:].bitcast(BF16)
                        nc.tensor.transpose(
                            pbf[:, :pt], xn[:pt, kk * P:(kk + 1) * P],
                            ident_bf[:pt, :pt])
                        nc.vector.tensor_scalar_mul(
                            dst1[:, kk, t0:t0 + pt], pbf[:, :pt],
                            g_sb[:, kk:kk + 1])
                        if dst2 is not None:
                            nc.vector.scalar_tensor_tensor(
                                dst2[:, kk, t0:t0 + pt], pbf[:, :pt],
                                g_sb[:, kk:kk + 1],
                                dst1[:, kk, t0:t0 + pt],
                                OP.mult, OP.subtract)

            def load_x(xb, b):
                nc.vector.memset(xb[64:, 4, :].bitcast(U32), 0)
                for ti, (t0, pt) in enumerate(TT):
                    rp = min(pt, S - t0)
                    nc.sync.dma_start(xb[:rp, ti], x_d[b, t0:t0 + rp, :])

            # ================= per-batch stages =================

            def stage_qkv(apool, xn1_fm, q8, k8, v_sb):
                # Q/K: feature-major out; V: token-major out (+ones col).
                nc.vector.memset(v_sb[:].bitcast(U32), 0)
                v_hc = v_sb[:].rearrange("p t (h c) -> p t h c", c=VS)
                nc.vector.tensor_copy(
                    v_hc[:, 0:4, :, 64:65],
                    onec8[:, 0:1, None, None].to_broadcast((P, 4, H, 1)))
                nc.vector.tensor_copy(
                    v_hc[:65, 4:5, :, 64:65],
                    onec8[:65, 0:1, None, None].to_broadcast((65, 1, H, 1)))

                for w_r, bias_sb, dst in ((wq_r, bq_sb, q8),
                                          (wk_r, bk_sb, k8)):
                    for blk in range(2):
                        wt = apool.tile([P, KK, 512], FP8, tag="wqkv",
                                        bufs=3, name="wqkv")
                        nc.sync.dma_start(
                            wt[:], w_r[:, :, blk * 512:(blk + 1) * 512])
                        for mi in range(4):
                            m = blk * 4 + mi
                            for (c0, cn) in NC:
                                ps = psA.tile([P, 512], F32, tag="pA")
                                for kp in range(4):
                                    nc.tensor.matmul(
                                        ps[:, :cn],
                                        wt[:, 2 * kp:2 * kp + 2,
                                           mi * P:(mi + 1) * P],
                                        xn1_fm[:, 2 * kp:2 * kp + 2,
                                               c0:c0 + cn],
                                        start=(kp == 0), stop=(kp == 3),
                                        perf_mode=DR)
                                nc.vector.tensor_scalar(
                                    dst[:, m, c0:c0 + cn], ps[:, :cn],
                                    1.0 / WSC, bias_sb[:, m:m + 1],
                                    OP.mult, OP.add)

                for blk in range(2):
                    wt = apool.tile([P, KK, 512], FP8, tag="wqkv",
                                    bufs=3, name="wqkv")
                    nc.sync.dma_start(
                        wt[:], wv_r[:, :, blk * 512:(blk + 1) * 512])
                    for ti, (t0, pt) in enumerate(TT):
                        rp = min(pt, S - t0)
                        for cc in range(2):
                            ci = blk * 2 + cc
                            c0 = ci * 256
                            ps = psA.tile([P, 512], F32, tag="pA")
                            for kp in range(4):
                                nc.tensor.matmul(
                                    ps[:pt, :256],
                                    xn1_fm[:, 2 * kp:2 * kp + 2, t0:t0 + pt],
                                    wt[:, 2 * kp:2 * kp + 2,
                                       cc * 256:(cc + 1) * 256],
                                    start=(kp == 0), stop=(kp == 3),
                                    perf_mode=DR)
                            nc.vector.scalar_tensor_tensor(
                                v_hc[:rp, ti, 4 * ci:4 * (ci + 1), 0:64],
                                ps[:rp, :256].rearrange(
                                    "p (h c) -> p h c", c=64),
                                1.0 / WSC,
                                bvb[:rp, c0:c0 + 256].rearrange(
                                    "p (h c) -> p h c", c=64),
                                OP.mult, OP.add)

            def emit_scores(h, q8, k8, es):
                # es[:, kt, q] = exp(q.k/8 - EXPB) in fp8
                hrow = (h % 2) * 64
                kkh = h // 2
                for (c0, cn) in SC:
                    for pair in ((0, 1), (2, 3), (4,)):
                        pg = psA.tile([P, 2, 512], F32, tag="pS", bufs=2,
                                      name="pg")
                        for j, kt in enumerate(pair):
                            t0, ptk = TT[kt]
                            nc.tensor.matmul(
                                pg[:ptk, j, :cn],
                                k8[hrow:hrow + 64, kkh, t0:t0 + ptk],
                                q8[hrow:hrow + 64, kkh, c0:c0 + cn],
                                start=True, stop=True)
                        npair = len(pair)
                        prow = TT[pair[0]][1]
                        nc.scalar.activation(
                            es[:prow, pair[0]:pair[0] + npair, c0:c0 + cn],
                            pg[:prow, :npair, :cn],
                            AF.Exp, scale=1.0 / np.sqrt(DH),
                            bias=nexpb[:prow])

            def emit_pv(h, es, v_sb, ctx_fm):
                hrow = (h % 2) * 64
                kkh = h // 2
                for (c0, cn) in NC:
                    pc = psA.tile([P, 512], F32, tag="pA")
                    for kp in range(2):
                        nc.tensor.matmul(
                            pc[:VS, :cn],
                            v_sb[:, 2 * kp:2 * kp + 2,
                                 h * VS:(h + 1) * VS],
                            es[:, 2 * kp:2 * kp + 2, c0:c0 + cn],
                            start=(kp == 0), stop=False, perf_mode=DR)
                    nc.tensor.matmul(
                        pc[:VS, :cn], v_sb[:66, 4, h * VS:(h + 1) * VS],
                        es[:66, 4, c0:c0 + cn], start=False, stop=True)
                    rc = lnpool.tile([1, 512], BF16, tag="rc", bufs=2)
                    with nc.allow_low_precision(reason="softmax 1/Z in bf16"):
                        nc.vector.reciprocal(rc[:, :cn], pc[64:65, :cn])
                    rb = lnpool.tile([64, 512], BF16, tag="rb", bufs=2)
                    nc.gpsimd.partition_broadcast(rb[:, :cn], rc[:, :cn])
                    nc.vector.scalar_tensor_tensor(
                        ctx_fm[hrow:hrow + 64, kkh, c0:c0 + cn],
                        pc[0:64, :cn], CTXSC, rb[:, :cn],
                        OP.mult, OP.mult)

            def stage_oproj(apool, ctx_fm, xb, x2, stats2):
                # out token-major: x2 = attn/(WSC*CTXSC) + x ; LN2 stats after
                for blk in range(2):
                    wt = apool.tile([P, KK, 512], FP8, tag="wqkv",
                                    bufs=3, name="wqkv")
                    nc.sync.dma_start(
                        wt[:], wo_r[:, :, blk * 512:(blk + 1) * 512])
                    for ti in (4, 0, 1, 2, 3):
                        t0, pt = TT[ti]
                        for cc in range(2):
                            ci = blk * 2 + cc
                            c0 = ci * 256
                            ps = psA.tile([P, 512], F32, tag="pA")
                            for kp in range(4):
                                nc.tensor.matmul(
                                    ps[:pt, :256],
                                    ctx_fm[:, 2 * kp:2 * kp + 2, t0:t0 + pt],
                                    wt[:, 2 * kp:2 * kp + 2,
                                       cc * 256:(cc + 1) * 256],
                                    start=(kp == 0), stop=False,
                                    perf_mode=DR)
                            nc.tensor.matmul(
                                ps[:pt, :256], ones_bf[:1, :pt],
                                bo_sb[:1, c0:c0 + 256],
                                start=False, stop=True)
                            nc.vector.scalar_tensor_tensor(
                                x2[:pt, ti, c0:c0 + 256], ps[:pt, :256],
                                1.0 / (WSC * CTXSC), xb[:pt, ti, c0:c0 + 256],
                                OP.mult, OP.add)
                for ti, (t0, pt) in enumerate(TT):
                    ln_tile_stats(stats2, x2, ti, pt)

            def stage_fc1(a1, a2, h1, h2, mpool, mwpool):
                for blk in range(8):
                    wa = mwpool.tile([P, KK, 512], FP8, tag="w1a", bufs=2)
                    nc.sync.dma_start(
                        wa[:], w1a_r[:, :, blk * 512:(blk + 1) * 512])
                    if FC1_TERMS == 3:
                        wb = mwpool.tile([P, KK, 512], FP8, tag="w1b",
                                         bufs=2)
                        nc.sync.dma_start(
                            wb[:], w1b_r[:, :, blk * 512:(blk + 1) * 512])
                    for mi in range(4):
                        m = blk * 4 + mi
                        mc = slice(mi * P, (mi + 1) * P)
                        hb = mpool.tile([P, SP], BF16, tag="hb", bufs=2)
                        for (c0, cn) in NC:
                            ps = psA.tile([P, 512], F32, tag="pA")
                            terms = [(a1, wa), (a2, wa)]
                            if FC1_TERMS == 3:
                                terms.append((a1, wb))
                            nterm = len(terms)
                            for tix, (asrc, wsrc) in enumerate(terms):
                                for kp in range(4):
                                    nc.tensor.matmul(
                                        ps[:, :cn],
                                        wsrc[:, 2 * kp:2 * kp + 2, mc],
                                        asrc[:, 2 * kp:2 * kp + 2,
                                             c0:c0 + cn],
                                        start=(tix == 0 and kp == 0),
                                        stop=(tix == nterm - 1 and kp == 3),
                                        perf_mode=DR)
                            nc.scalar.activation(
                                h1[:, m, c0:c0 + cn], ps[:, :cn], _GELU,
                                bias=b1_sb[:, m:m + 1], scale=1.0 / WSC)
                            nc.scalar.activation(
                                hb[:, c0:c0 + cn], ps[:, :cn], _GELU,
                                bias=b1_sb[:, m:m + 1], scale=1.0 / WSC)
                        nc.gpsimd.tensor_tensor(
                            h2[:, m, :], hb[:, :], h1[:, m, :], OP.subtract)

            def stage_fc2(h1, h2, x2, y_sb, mpool, mwpool):
                for blk in range(4):
                    wa = mwpool.tile([P, FK, 256], FP8, tag="w2a", bufs=2)
                    nc.sync.dma_start(
                        wa[:], w2a_r[:, :, blk * 256:(blk + 1) * 256])
                    if FC2_TERMS == 3:
                        wb = mwpool.tile([P, FK, 256], FP8, tag="w2b",
                                         bufs=2)
                        nc.sync.dma_start(
                            wb[:], w2b_r[:, :, blk * 256:(blk + 1) * 256])
                    for mi in range(2):
                        m = blk * 2 + mi
                        mc = slice(mi * P, (mi + 1) * P)
                        mlp_fm = mpool.tile([P, SP], BF16, tag="mlp_fm",
                                            bufs=2)
                        for (c0, cn) in NC:
                            ps = psA.tile([P, 512], F32, tag="pA")
                            terms = [(h1, wa), (h2, wa)]
                            if FC2_TERMS == 3:
                                terms.append((h1, wb))
                            nterm = len(terms)
                            for tix, (hsrc, wsrc) in enumerate(terms):
                                for kp in range(FK // 2):
                                    nc.tensor.matmul(
                                        ps[:, :cn],
                                        wsrc[:, 2 * kp:2 * kp + 2, mc],
                                        hsrc[:, 2 * kp:2 * kp + 2,
                                             c0:c0 + cn],
                                        start=(tix == 0 and kp == 0),
                                        stop=(tix == nterm - 1 and
                                              kp == FK // 2 - 1),
                                        perf_mode=DR)
                            nc.vector.tensor_scalar(
                                mlp_fm[:, c0:c0 + cn], ps[:, :cn],
                                1.0 / W2SC, b2_sb[:, m:m + 1],
                                OP.mult, OP.add)
                        # transpose back + residual into y staging
                        for ti, (t0, pt) in enumerate(TT):
                            pst = psA.tile([P, 512], F32, tag="pA")
                            pbf = pst[:].bitcast(BF16)
                            nc.tensor.transpose(
                                pbf[:pt, :P], mlp_fm[:, t0:t0 + pt],
                                ident_bf[:])
                            nc.vector.scalar_tensor_tensor(
                                y_sb[:pt, ti, m * P:(m + 1) * P],
                                pbf[:pt, :P], 0.0,
                                x2[:pt, ti, m * P:(m + 1) * P],
                                OP.add, OP.add)

            # ================= batch loop =================
            for b in range(BL):
                xb = rpool.tile([P, 5, D], BF16, tag="xb", bufs=2)
                load_x(xb, b)
                stats1 = ln_new_stats()
                for ti, (t0, pt) in enumerate(TT):
                    ln_tile_stats(stats1, xb, ti, pt)

                xn1_fm = fmpool.tile([P, KK, SP], FP8, tag="xn1_fm")
                ln_finalize(stats1, 0, 2)
                ln_apply_tiles(stats1, xb, g1_sb, xn1_fm, None, (0, 1))
                ln_finalize(stats1, 2, 5)
                ln_apply_tiles(stats1, xb, g1_sb, xn1_fm, None, (2, 3, 4))

                x2 = rpool.tile([P, 5, D], BF16, tag="x2", bufs=1)
                with tc.tile_pool(name="attn", bufs=1) as apool:
                    q8 = apool.tile([P, KK, SP], FP8, tag="q8")
                    k8 = apool.tile([P, KK, SP], FP8, tag="k8")
                    v_sb = apool.tile([P, 5, H * VS], FP8, tag="v")
                    ctx_fm = apool.tile([P, KK, SP], FP8, tag="ctx")

                    stage_qkv(apool, xn1_fm, q8, k8, v_sb)

                    def get_es(i):
                        return apool.tile([P, 5, SP], FP8, tag=f"es{i % 2}",
                                          name=f"es{i % 2}")

                    es_cur = get_es(0)
                    emit_scores(0, q8, k8, es_cur)
                    for h in range(H):
                        if h + 1 < H:
                            es_nxt = get_es(h + 1)
                            emit_scores(h + 1, q8, k8, es_nxt)
                        emit_pv(h, es_cur, v_sb, ctx_fm)
                        if h + 1 < H:
                            es_cur = es_nxt

                    stats2 = ln_new_stats()
                    stage_oproj(apool, ctx_fm, xb, x2, stats2)

                # ---- LN2 -> a1 (+a2 residual split) ----
                a1 = fmpool.tile([P, KK, SP], FP8, tag="a1")
                a2 = fmpool.tile([P, KK, SP], FP8, tag="a2")
                ln_finalize(stats2, 0, 5)
                ln_apply_tiles(stats2, x2, g2_sb, a1, a2, (0, 1, 2, 3, 4))

                y_sb = ypool.tile([P, 5, D], BF16, tag="y_sb")
                with tc.tile_pool(name="mlp", bufs=1) as mpool, \
                     tc.tile_pool(name="wmlp", bufs=1) as mwpool:
                    h1 = mpool.tile([P, FK, SP], FP8, tag="h1")
                    h2 = mpool.tile([P, FK, SP], FP8, tag="h2")
                    stage_fc1(a1, a2, h1, h2, mpool, mwpool)
                    stage_fc2(h1, h2, x2, y_sb, mpool, mwpool)

                for ti, (t0, pt) in enumerate(TT):
                    rp = min(pt, S - t0)
                    nc.sync.dma_start(y_d[b, t0:t0 + rp, :], y_sb[:rp, ti])

    nc.compile()
    return nc


def _get_nc():
    global _NC_CACHE
    if _NC_CACHE is None:
        _NC_CACHE = _build()
    return _NC_CACHE


def kernel(**inputs):
    nc = _get_nc()
    shared = prepare_shared(inputs)
    x = np.asarray(inputs["x"], np.float32).astype(BFNP)
    in_maps = []
    for i in range(NCORES):
        m = dict(shared)
        m["x"] = np.ascontiguousarray(x[i * BL:(i + 1) * BL])
        in_maps.append(m)
    res = bass_utils.run_bass_kernel_spmd(nc, in_maps,
                                          core_ids=list(range(NCORES)))
    y = np.concatenate([np.asarray(res.results[i]["y"])
                        for i in range(NCORES)], axis=0)
    return y.astype(np.float32)


# revision 8
# speedup vs baseline: 1.1731x; 1.0167x over previous
"""Trainium2 Bass kernel for a dense transformer block (pre-LN attention +
GELU MLP) — fp8e4m3 DoubleRow edition.

Strategy: data-parallel over batch across 8 NeuronCores (2 batches/core, no
collectives).  All matmuls run in fp8e4m3 with MatmulPerfMode.DoubleRow
(2 k-tiles per instruction at 0.5 cycles/row = 4x the fp32r rate).  Accuracy
is held by:
  - weights pre-scaled by 64/128 into fp8's normal range (inverse scale is
    folded into the free scalar slots of psum->SBUF copy ops),
  - residual-split operands: the MLP input and hidden activations are
    represented as main+residual fp8 pairs (a1+a2, h1+h2), and w1/w2 carry a
    matched-scale fp8 residual term (w1b/w2b), so the MLP is computed to
    ~0.3% while still running entirely at DoubleRow rate,
  - softmax without max-subtraction: es = exp(s - 3.2) stored in fp8 (the
    constant bias cancels in the normalization; max score ~8.2 so es < 240).
LayerNorm beta terms are folded into the following layer's biases on the host
(exactly linear), so the on-chip LN applies only the gain.

Engine-overhead notes: psum accumulation groups span disjoint column
sub-chunks of one bank (start zeroes the whole 2KB region), so ACT/DVE
psum->SBUF ops cover 512-col blocks; the 8 per-tile LN transposes land in one
bank (8 x 128 bf16 = 2KB) and are applied with a single DVE op; gelu runs one
ACT pass to bf16 with the fp8 split produced on the (otherwise idle) GpSimd.
Streamed weights are pre-blocked on the host so every DMA descriptor is a
contiguous 4-8KB per-partition run.
"""

import numpy as np
import ml_dtypes

import concourse.bass as bass
import concourse.mybir as mybir
import concourse.tile as tile
from concourse import bacc, bass_utils
from concourse.masks import make_identity

# Problem shape (hardcoded per spec nn_Block_58652073394865)
B, S, D, H, F = 16, 577, 1024, 16, 4096
DH = D // H
NCORES = 8
BL = B // NCORES        # batches per core
P = 128
KK = D // P             # 8 chunks of the model dim
FK = F // P             # 32 chunks of the mlp dim
EPS = 1e-6

SP = 578                # tokens padded with one zero token
TT = [(0, 128), (128, 128), (256, 128), (384, 128), (512, 66)]
# psum groups: [0,512) as two 256-wide DR sub-chunks in one bank, then [512,578)
CH = [(0, 512, (256, 256)), (512, 66, (66,))]
SC = [(0, 512), (512, 66)]               # plain-fp8 score chunks (out <= 512)
VS = 66                 # per-head stride in v (64 v + 1 ones + 1 pad)

WSC = 64.0              # fp8 pre-scale for wq/wk/wv/wo/w1
W2SC = 128.0            # fp8 pre-scale for w2
CTXSC = 16.0            # fp8 pre-scale for ctx
EXPB = 3.2              # softmax exp bias (cancels in normalization)
FC1_TERMS = 3           # 2: a1@w1a + a2@w1a;  3: + a1@w1b
FC2_TERMS = 3           # 2: h1@w2a + h2@w2a;  3: + h1@w2b

F32 = mybir.dt.float32
BF16 = mybir.dt.bfloat16
FP8 = mybir.dt.float8e4
U32 = mybir.dt.uint32
AF = mybir.ActivationFunctionType
OP = mybir.AluOpType
DR = mybir.MatmulPerfMode.DoubleRow

E4NP = ml_dtypes.float8_e4m3
BFNP = ml_dtypes.bfloat16

_NC_CACHE = None
# CoreSim doesn't implement the Gelu LUT; tests may swap this for AF.Tanh
_GELU = AF.Gelu

SHARED_NAMES = ["wq", "wk", "wv", "wo", "w1a", "w2a", "bq", "bk", "bv", "bo",
                "b1", "b2", "g1", "g2"]
if FC1_TERMS == 3:
    SHARED_NAMES.append("w1b")
if FC2_TERMS == 3:
    SHARED_NAMES.append("w2b")


def _block_layout(w8, nblk, cols):
    """[K, N] fp8 -> [nblk, 128, (K//128) * cols] with per-partition
    contiguous (ko, col) runs, matching SBUF tiles [P, K//128, cols]."""
    Kd, Nd = w8.shape
    ko = Kd // P
    # arr[b, p, k, c] = w8[k*128 + p, b*cols + c]
    a = w8.reshape(ko, P, nblk, cols).transpose(2, 1, 0, 3)
    return np.ascontiguousarray(a.reshape(nblk, P, ko * cols))


def prepare_shared(inputs):
    """Host-side: quantize/scale weights, fold LN betas into biases."""
    f = {n: np.ascontiguousarray(np.asarray(inputs[n], np.float32))
         for n in ("wq", "wk", "wv", "wo", "w1", "w2", "bq", "bk", "bv", "bo",
                   "b1", "b2", "ln1_g", "ln1_b", "ln2_g", "ln2_b")}

    def q8s(w, s):
        return (w * s).astype(E4NP)

    out = {
        "wq": _block_layout(q8s(f["wq"], WSC), 2, 512),
        "wk": _block_layout(q8s(f["wk"], WSC), 2, 512),
        "wv": _block_layout(q8s(f["wv"], WSC), 2, 512),
        "wo": _block_layout(q8s(f["wo"], WSC), 2, 512),
        "g1": f["ln1_g"], "g2": f["ln2_g"],
        "bq": f["bq"] + f["ln1_b"] @ f["wq"],
        "bk": f["bk"] + f["ln1_b"] @ f["wk"],
        "bv": f["bv"] + f["ln1_b"] @ f["wv"],
        "bo": np.ascontiguousarray(
            (f["bo"] * (WSC * CTXSC)).astype(BFNP)),
        "b1": f["b1"] + f["ln2_b"] @ f["w1"],
        "b2": f["b2"],
    }
    w1s = f["w1"] * WSC
    w1a = w1s.astype(E4NP)
    out["w1a"] = _block_layout(w1a, 8, 512)
    if FC1_TERMS == 3:
        out["w1b"] = _block_layout(
            (w1s - w1a.astype(np.float32)).astype(E4NP), 8, 512)
    w2s = f["w2"] * W2SC
    w2a = w2s.astype(E4NP)
    out["w2a"] = _block_layout(w2a, 4, 256)
    if FC2_TERMS == 3:
        out["w2b"] = _block_layout(
            (w2s - w2a.astype(np.float32)).astype(E4NP), 4, 256)
    return out


def _build():
    nc = bacc.Bacc("TRN2", target_bir_lowering=False, debug=False,
                   num_devices=NCORES)

    x_d = nc.dram_tensor("x", [BL, S, D], BF16, kind="ExternalInput").ap()
    y_d = nc.dram_tensor("y", [BL, S, D], BF16, kind="ExternalOutput").ap()
    wq_d = nc.dram_tensor("wq", [2, P, KK * 512], FP8, kind="ExternalInput").ap()
    wk_d = nc.dram_tensor("wk", [2, P, KK * 512], FP8, kind="ExternalInput").ap()
    wv_d = nc.dram_tensor("wv", [2, P, KK * 512], FP8, kind="ExternalInput").ap()
    wo_d = nc.dram_tensor("wo", [2, P, KK * 512], FP8, kind="ExternalInput").ap()
    w1a_d = nc.dram_tensor("w1a", [8, P, KK * 512], FP8, kind="ExternalInput").ap()
    w2a_d = nc.dram_tensor("w2a", [4, P, FK * 256], FP8, kind="ExternalInput").ap()
    w1b_d = (nc.dram_tensor("w1b", [8, P, KK * 512], FP8,
                            kind="ExternalInput").ap()
             if FC1_TERMS == 3 else None)
    w2b_d = (nc.dram_tensor("w2b", [4, P, FK * 256], FP8,
                            kind="ExternalInput").ap()
             if FC2_TERMS == 3 else None)
    bq_d = nc.dram_tensor("bq", [D], F32, kind="ExternalInput").ap()
    bk_d = nc.dram_tensor("bk", [D], F32, kind="ExternalInput").ap()
    bv_d = nc.dram_tensor("bv", [D], F32, kind="ExternalInput").ap()
    bo_d = nc.dram_tensor("bo", [D], BF16, kind="ExternalInput").ap()
    b1_d = nc.dram_tensor("b1", [F], F32, kind="ExternalInput").ap()
    b2_d = nc.dram_tensor("b2", [D], F32, kind="ExternalInput").ap()
    g1_d = nc.dram_tensor("g1", [D], F32, kind="ExternalInput").ap()
    g2_d = nc.dram_tensor("g2", [D], F32, kind="ExternalInput").ap()

    with tile.TileContext(nc) as tc:
        with tc.tile_pool(name="const", bufs=1) as cpool, \
             tc.tile_pool(name="resid", bufs=1) as rpool, \
             tc.tile_pool(name="fmbuf", bufs=1) as fmpool, \
             tc.tile_pool(name="lnp", bufs=2) as lnpool, \
             tc.tile_pool(name="ystg", bufs=1) as ypool, \
             tc.tile_pool(name="psA", bufs=4, space="PSUM") as psA:

            # ---- small params, packed ----
            cA = cpool.tile([P, 5 * KK + FK], F32, tag="cA")
            bq_sb = cA[:, 0:KK]
            bk_sb = cA[:, KK:2 * KK]
            b2_sb = cA[:, 2 * KK:3 * KK]
            g1_sb = cA[:, 3 * KK:4 * KK]
            g2_sb = cA[:, 4 * KK:5 * KK]
            b1_sb = cA[:, 5 * KK:5 * KK + FK]
            nc.sync.dma_start(bq_sb, bq_d.rearrange("(m p) -> p m", p=P))
            nc.sync.dma_start(bk_sb, bk_d.rearrange("(m p) -> p m", p=P))
            nc.sync.dma_start(b2_sb, b2_d.rearrange("(m p) -> p m", p=P))
            nc.sync.dma_start(g1_sb, g1_d.rearrange("(c p) -> p c", p=P))
            nc.sync.dma_start(g2_sb, g2_d.rearrange("(c p) -> p c", p=P))
            nc.sync.dma_start(b1_sb, b1_d.rearrange("(m p) -> p m", p=P))

            cB = cpool.tile([P, P + 3], F32, tag="cB")
            ident_f = cB[:, 0:P]
            epsap = cB[:, P:P + 1]
            onef = cB[:, P + 1:P + 2]
            nexpb = cB[:, P + 2:P + 3]
            make_identity(nc, ident_f)
            nc.vector.memset(epsap, EPS)
            nc.vector.memset(onef, 1.0)
            nc.vector.memset(nexpb, -EXPB)

            cC = cpool.tile([P, P + 4], BF16, tag="cC")
            ident_bf = cC[:, 0:P]
            nc.vector.tensor_copy(ident_bf, ident_f)
            cD = cpool.tile([1, P + D], BF16, tag="cD")
            ones_bf = cD[:, 0:P]
            bo_sb = cD[:, P:P + D]
            nc.vector.memset(ones_bf, 1.0)
            nc.sync.dma_start(bo_sb, bo_d[None, :])

            onec8 = cpool.tile([P, 4], FP8, tag="onec8")
            nc.vector.tensor_copy(onec8[:, 0:1], onef)

            # bv broadcast to all partitions (bias varies along free dim)
            bvrow = cpool.tile([1, D], F32, tag="bvrow")
            nc.sync.dma_start(bvrow[:], bv_d[None, :])
            bvb = cpool.tile([P, D], F32, tag="bvb")
            nc.gpsimd.partition_broadcast(bvb[:], bvrow[:])

            # ---- LayerNorm helpers (token-major stats; gain-only apply) ----
            def ln_new_stats():
                stats = lnpool.tile([P, 20], F32, tag="stats")
                nc.vector.memset(stats[:, 0:5], 0.0)
                nc.vector.memset(stats[:, 5:10], 1.0)
                return stats

            def ln_tile_stats(stats, src, ti, pt):
                negmu = stats[:, 0:5]
                varD = stats[:, 5:10]
                nc.vector.tensor_reduce(
                    negmu[:pt, ti:ti + 1], src[:pt, ti],
                    mybir.AxisListType.X, OP.add)
                nc.vector.tensor_scalar_mul(
                    negmu[:pt, ti:ti + 1], negmu[:pt, ti:ti + 1], -1.0 / D)
                scr = lnpool.tile([P, D], BF16, tag="xsq", bufs=2)
                nc.scalar.activation(
                    scr[:pt], src[:pt, ti], AF.Square,
                    bias=negmu[:pt, ti:ti + 1], accum_out=varD[:pt, ti:ti + 1])

            def ln_finalize(stats, lo, hi):
                nc.scalar.activation(stats[:, 10 + lo:10 + hi],
                                     stats[:, 5 + lo:5 + hi], AF.Sqrt,
                                     scale=1.0 / D, bias=epsap[:])
                nc.vector.reciprocal(stats[:, 15 + lo:15 + hi],
                                     stats[:, 10 + lo:10 + hi])

            def ln_apply_tiles(stats, src, g_sb, dst1, dst2, tis):
                # normalize (bf16), 8 PE-transposes into ONE psum bank, then
                # a single DVE gain-apply per tile (to_broadcast per-kk gain).
                # dst2 (if not None) receives the fp8 residual split.
                negmu = stats[:, 0:5]
                rsig = stats[:, 15:20]
                gb = g_sb[:, :, None]
                for ti in tis:
                    t0, pt = TT[ti]
                    xn = lnpool.tile([P, D], BF16, tag="xn_tm", bufs=2)
                    nc.vector.tensor_scalar(
                        xn[:pt], src[:pt, ti],
                        negmu[:pt, ti:ti + 1], rsig[:pt, ti:ti + 1],
                        OP.add, OP.mult)
                    pst = psA.tile([P, 512], F32, tag="pA")
                    pbf = pst[:].bitcast(BF16).rearrange(
                        "p (k c) -> p k c", c=P)
                    for kk in range(KK):
                        nc.tensor.matmul(
                            pbf[:, kk, :pt] if pt == P else pbf[:, kk, :pt],
                            xn[:pt, kk * P:(kk + 1) * P],
                            ident_bf[:pt, :pt],
                            is_transpose=True,
                            start=(kk == 0), stop=(kk == KK - 1))
                    if dst2 is None:
                        nc.vector.tensor_tensor(
                            dst1[:, :, t0:t0 + pt], pbf[:, :, :pt],
                            gb.to_broadcast((P, KK, pt)), OP.mult)
                    else:
                        xp = lnpool.tile([P, KK, P], BF16, tag="xprod",
                                         bufs=2)
                        nc.vector.tensor_tensor(
                            xp[:, :, :pt], pbf[:, :, :pt],
                            gb.to_broadcast((P, KK, pt)), OP.mult)
                        nc.vector.tensor_copy(
                            dst1[:, :, t0:t0 + pt], xp[:, :, :pt])
                        nc.vector.tensor_tensor(
                            dst2[:, :, t0:t0 + pt], xp[:, :, :pt],
                            dst1[:, :, t0:t0 + pt], OP.subtract)

            def load_x(xb, b):
                nc.vector.memset(xb[64:, 4, :].bitcast(U32), 0)
                for ti, (t0, pt) in enumerate(TT):
                    rp = min(pt, S - t0)
                    nc.sync.dma_start(xb[:rp, ti], x_d[b, t0:t0 + rp, :])

            # ================= per-batch stages =================

            def stage_qkv(apool, xn1_fm, q8, k8, v_sb):
                # Q/K: feature-major out; V: token-major out (+ones col).
                nc.vector.memset(v_sb[:].bitcast(U32), 0)
                v_hc = v_sb[:].rearrange("p t (h c) -> p t h c", c=VS)
                nc.vector.tensor_copy(
                    v_hc[:, 0:4, :, 64:65],
                    onec8[:, 0:1, None, None].to_broadcast((P, 4, H, 1)))
                nc.vector.tensor_copy(
                    v_hc[:65, 4:5, :, 64:65],
                    onec8[:65, 0:1, None, None].to_broadcast((65, 1, H, 1)))

                for w_d, bias_sb, dst in ((wq_d, bq_sb, q8),
                                          (wk_d, bk_sb, k8)):
                    for blk in range(2):
                        wt = apool.tile([P, KK, 512], FP8, tag="wqkv",
                                        bufs=3, name="wqkv")
                        nc.sync.dma_start(
                            wt[:], w_d[blk].rearrange("p (k c) -> p k c",
                                                      c=512))
                        for mi in range(4):
                            m = blk * 4 + mi
                            for (c0, cw, subs) in CH:
                                ps = psA.tile([P, 512], F32, tag="pA")
                                nsub = len(subs)
                                for si in range(nsub):
                                    s0 = c0 + si * 256
                                    sn = subs[si]
                                    for kp in range(4):
                                        nc.tensor.matmul(
                                            ps[:, si * 256:si * 256 + sn],
                                            wt[:, 2 * kp:2 * kp + 2,
                                               mi * P:(mi + 1) * P],
                                            xn1_fm[:, 2 * kp:2 * kp + 2,
                                                   s0:s0 + sn],
                                            start=(si == 0 and kp == 0),
                                            stop=(si == nsub - 1 and kp == 3),
                                            perf_mode=DR)
                                nc.vector.tensor_scalar(
                                    dst[:, m, c0:c0 + cw], ps[:, :cw],
                                    1.0 / WSC, bias_sb[:, m:m + 1],
                                    OP.mult, OP.add)

                for blk in range(2):
                    wt = apool.tile([P, KK, 512], FP8, tag="wqkv",
                                    bufs=3, name="wqkv")
                    nc.sync.dma_start(
                        wt[:], wv_d[blk].rearrange("p (k c) -> p k c", c=512))
                    for ti, (t0, pt) in enumerate(TT):
                        rp = min(pt, S - t0)
                        ps = psA.tile([P, 512], F32, tag="pA")
                        for cc in range(2):
                            for kp in range(4):
                                nc.tensor.matmul(
                                    ps[:pt, cc * 256:(cc + 1) * 256],
                                    xn1_fm[:, 2 * kp:2 * kp + 2, t0:t0 + pt],
                                    wt[:, 2 * kp:2 * kp + 2,
                                       cc * 256:(cc + 1) * 256],
                                    start=(cc == 0 and kp == 0),
                                    stop=(cc == 1 and kp == 3),
                                    perf_mode=DR)
                        c0 = blk * 512
                        nc.vector.scalar_tensor_tensor(
                            v_hc[:rp, ti, 8 * blk:8 * (blk + 1), 0:64],
                            ps[:rp, :512].rearrange("p (h c) -> p h c", c=64),
                            1.0 / WSC,
                            bvb[:rp, c0:c0 + 512].rearrange(
                                "p (h c) -> p h c", c=64),
                            OP.mult, OP.add)

            def emit_scores(h, q8, k8, es):
                # es[:, kt, q] = exp(q.k/8 - EXPB) in fp8
                hrow = (h % 2) * 64
                kkh = h // 2
                for (c0, cn) in SC:
                    for pair in ((0, 1), (2, 3), (4,)):
                        pg = psA.tile([P, 2, 512], F32, tag="pS", bufs=2,
                                      name="pg")
                        for j, kt in enumerate(pair):
                            t0, ptk = TT[kt]
                            nc.tensor.matmul(
                                pg[:ptk, j, :cn],
                                k8[hrow:hrow + 64, kkh, t0:t0 + ptk],
                                q8[hrow:hrow + 64, kkh, c0:c0 + cn],
                                start=True, stop=True)
                        npair = len(pair)
                        prow = TT[pair[0]][1]
                        nc.scalar.activation(
                            es[:prow, pair[0]:pair[0] + npair, c0:c0 + cn],
                            pg[:prow, :npair, :cn],
                            AF.Exp, scale=1.0 / np.sqrt(DH),
                            bias=nexpb[:prow])

            def emit_pv(h, es, v_sb, ctx_fm):
                hrow = (h % 2) * 64
                kkh = h // 2
                for (c0, cw, subs) in CH:
                    pc = psA.tile([P, 512], F32, tag="pA")
                    nsub = len(subs)
                    for si in range(nsub):
                        s0 = c0 + si * 256
                        sn = subs[si]
                        for kp in range(2):
                            nc.tensor.matmul(
                                pc[:VS, si * 256:si * 256 + sn],
                                v_sb[:, 2 * kp:2 * kp + 2,
                                     h * VS:(h + 1) * VS],
                                es[:, 2 * kp:2 * kp + 2, s0:s0 + sn],
                                start=(si == 0 and kp == 0), stop=False,
                                perf_mode=DR)
                        nc.tensor.matmul(
                            pc[:VS, si * 256:si * 256 + sn],
                            v_sb[:66, 4, h * VS:(h + 1) * VS],
                            es[:66, 4, s0:s0 + sn],
                            start=False, stop=(si == nsub - 1))
                    rc = lnpool.tile([1, 512], BF16, tag="rc", bufs=2)
                    with nc.allow_low_precision(reason="softmax 1/Z bf16"):
                        nc.vector.reciprocal(rc[:, :cw], pc[64:65, :cw])
                    rb = lnpool.tile([64, 512], BF16, tag="rb", bufs=2)
                    nc.gpsimd.partition_broadcast(rb[:, :cw], rc[:, :cw])
                    nc.vector.scalar_tensor_tensor(
                        ctx_fm[hrow:hrow + 64, kkh, c0:c0 + cw],
                        pc[0:64, :cw], CTXSC, rb[:, :cw],
                        OP.mult, OP.mult)

            def stage_oproj(apool, ctx_fm, xb, x2, stats2):
                # out token-major: x2 = attn/(WSC*CTXSC) + x ; LN2 stats after
                for blk in range(2):
                    wt = apool.tile([P, KK, 512], FP8, tag="wqkv",
                                    bufs=3, name="wqkv")
                    nc.sync.dma_start(
                        wt[:], wo_d[blk].rearrange("p (k c) -> p k c", c=512))
                    c0 = blk * 512
                    for ti in (4, 0, 1, 2, 3):
                        t0, pt = TT[ti]
                        ps = psA.tile([P, 512], F32, tag="pA")
                        for cc in range(2):
                            for kp in range(4):
                                nc.tensor.matmul(
                                    ps[:pt, cc * 256:(cc + 1) * 256],
                                    ctx_fm[:, 2 * kp:2 * kp + 2, t0:t0 + pt],
                                    wt[:, 2 * kp:2 * kp + 2,
                                       cc * 256:(cc + 1) * 256],
                                    start=(cc == 0 and kp == 0), stop=False,
                                    perf_mode=DR)
                        nc.tensor.matmul(
                            ps[:pt, :512], ones_bf[:1, :pt],
                            bo_sb[:1, c0:c0 + 512], start=False, stop=True)
                        nc.vector.scalar_tensor_tensor(
                            x2[:pt, ti, c0:c0 + 512], ps[:pt, :512],
                            1.0 / (WSC * CTXSC), xb[:pt, ti, c0:c0 + 512],
                            OP.mult, OP.add)
                for ti, (t0, pt) in enumerate(TT):
                    ln_tile_stats(stats2, x2, ti, pt)

            def stage_fc1(a1, a2, h1, h2, mpool, mwpool):
                for blk in range(8):
                    wa = mwpool.tile([P, KK, 512], FP8, tag="w1a", bufs=2)
                    nc.sync.dma_start(
                        wa[:], w1a_d[blk].rearrange("p (k c) -> p k c",
                                                    c=512))
                    if FC1_TERMS == 3:
                        wb = mwpool.tile([P, KK, 512], FP8, tag="w1b",
                                         bufs=2)
                        nc.sync.dma_start(
                            wb[:], w1b_d[blk].rearrange("p (k c) -> p k c",
                                                        c=512))
                    for mi in range(4):
                        m = blk * 4 + mi
                        mc = slice(mi * P, (mi + 1) * P)
                        hb = mpool.tile([P, SP], BF16, tag="hb", bufs=2)
                        for (c0, cw, subs) in CH:
                            ps = psA.tile([P, 512], F32, tag="pA")
                            terms = [(a1, wa), (a2, wa)]
                            if FC1_TERMS == 3:
                                terms.append((a1, wb))
                            nterm = len(terms)
                            nsub = len(subs)
                            for si in range(nsub):
                                s0 = c0 + si * 256
                                sn = subs[si]
                                for tix, (asrc, wsrc) in enumerate(terms):
                                    for kp in range(4):
                                        nc.tensor.matmul(
                                            ps[:, si * 256:si * 256 + sn],
                                            wsrc[:, 2 * kp:2 * kp + 2, mc],
                                            asrc[:, 2 * kp:2 * kp + 2,
                                                 s0:s0 + sn],
                                            start=(si == 0 and tix == 0
                                                   and kp == 0),
                                            stop=(si == nsub - 1 and
                                                  tix == nterm - 1 and
                                                  kp == 3),
                                            perf_mode=DR)
                            nc.scalar.activation(
                                hb[:, c0:c0 + cw], ps[:, :cw], _GELU,
                                bias=b1_sb[:, m:m + 1], scale=1.0 / WSC)
                        nc.gpsimd.tensor_copy(h1[:, m, :], hb[:, :])
                        nc.gpsimd.tensor_tensor(
                            h2[:, m, :], hb[:, :], h1[:, m, :], OP.subtract)

            def stage_fc2(h1, h2, x2, y_sb, mpool, mwpool):
                for blk in range(4):
                    wa = mwpool.tile([P, FK, 256], FP8, tag="w2a", bufs=2)
                    nc.sync.dma_start(
                        wa[:], w2a_d[blk].rearrange("p (k c) -> p k c",
                                                    c=256))
                    if FC2_TERMS == 3:
                        wb = mwpool.tile([P, FK, 256], FP8, tag="w2b",
                                         bufs=2)
                        nc.sync.dma_start(
                            wb[:], w2b_d[blk].rearrange("p (k c) -> p k c",
                                                        c=256))
                    for mi in range(2):
                        m = blk * 2 + mi
                        mc = slice(mi * P, (mi + 1) * P)
                        mlp_fm = mpool.tile([P, SP], BF16, tag="mlp_fm",
                                            bufs=2)
                        for (c0, cw, subs) in CH:
                            ps = psA.tile([P, 512], F32, tag="pA")
                            terms = [(h1, wa), (h2, wa)]
                            if FC2_TERMS == 3:
                                terms.append((h1, wb))
                            nterm = len(terms)
                            nsub = len(subs)
                            for si in range(nsub):
                                s0 = c0 + si * 256
                                sn = subs[si]
                                for tix, (hsrc, wsrc) in enumerate(terms):
                                    for kp in range(FK // 2):
                                        nc.tensor.matmul(
                                            ps[:, si * 256:si * 256 + sn],
                                            wsrc[:, 2 * kp:2 * kp + 2, mc],
                                            hsrc[:, 2 * kp:2 * kp + 2,
                                                 s0:s0 + sn],
                                            start=(si == 0 and tix == 0
                                                   and kp == 0),
                                            stop=(si == nsub - 1 and
                                                  tix == nterm - 1 and
                                                  kp == FK // 2 - 1),
                                            perf_mode=DR)
                            nc.vector.tensor_scalar(
                                mlp_fm[:, c0:c0 + cw], ps[:, :cw],
                                1.0 / W2SC, b2_sb[:, m:m + 1],
                                OP.mult, OP.add)
                        # 5 transposes into one bank; single residual op.
                        # (tile0's start zeroes the whole region, so the
                        # garbage rows of tile4 read as zeros.)
                        pst = psA.tile([P, 512], F32, tag="pA")
                        pbf = pst[:].bitcast(BF16).rearrange(
                            "p (t c) -> p t c", c=P)
                        for ti, (t0, pt) in enumerate(TT):
                            nc.tensor.matmul(
                                pbf[:pt, ti, :P], mlp_fm[:, t0:t0 + pt],
                                ident_bf[:], is_transpose=True,
                                start=(ti == 0), stop=(ti == 4))
                        nc.vector.scalar_tensor_tensor(
                            y_sb[:, :, m * P:(m + 1) * P],
                            pbf[:, 0:5, :], 0.0,
                            x2[:, :, m * P:(m + 1) * P],
                            OP.add, OP.add)

            # ================= batch loop =================
            for b in range(BL):
                xb = rpool.tile([P, 5, D], BF16, tag="xb", bufs=2)
                load_x(xb, b)
                stats1 = ln_new_stats()
                for ti, (t0, pt) in enumerate(TT):
                    ln_tile_stats(stats1, xb, ti, pt)

                xn1_fm = fmpool.tile([P, KK, SP], FP8, tag="xn1_fm")
                ln_finalize(stats1, 0, 2)
                ln_apply_tiles(stats1, xb, g1_sb, xn1_fm, None, (0, 1))
                ln_finalize(stats1, 2, 5)
                ln_apply_tiles(stats1, xb, g1_sb, xn1_fm, None, (2, 3, 4))

                x2 = rpool.tile([P, 5, D], BF16, tag="x2", bufs=1)
                with tc.tile_pool(name="attn", bufs=1) as apool:
                    q8 = apool.tile([P, KK, SP], FP8, tag="q8")
                    k8 = apool.tile([P, KK, SP], FP8, tag="k8")
                    v_sb = apool.tile([P, 5, H * VS], FP8, tag="v")
                    ctx_fm = apool.tile([P, KK, SP], FP8, tag="ctx")

                    stage_qkv(apool, xn1_fm, q8, k8, v_sb)

                    def get_es(i):
                        return apool.tile([P, 5, SP], FP8, tag=f"es{i % 2}",
                                          name=f"es{i % 2}")

                    es_cur = get_es(0)
                    emit_scores(0, q8, k8, es_cur)
                    for h in range(H):
                        if h + 1 < H:
                            es_nxt = get_es(h + 1)
                            emit_scores(h + 1, q8, k8, es_nxt)
                        emit_pv(h, es_cur, v_sb, ctx_fm)
                        if h + 1 < H:
                            es_cur = es_nxt

                    stats2 = ln_new_stats()
                    stage_oproj(apool, ctx_fm, xb, x2, stats2)

                # ---- LN2 -> a1 (+a2 residual split) ----
                a1 = fmpool.tile([P, KK, SP], FP8, tag="a1")
                a2 = fmpool.tile([P, KK, SP], FP8, tag="a2")
                ln_finalize(stats2, 0, 5)
                ln_apply_tiles(stats2, x2, g2_sb, a1, a2, (0, 1, 2, 3, 4))

                y_sb = ypool.tile([P, 5, D], BF16, tag="y_sb")
                with tc.tile_pool(name="mlp", bufs=1) as mpool, \
                     tc.tile_pool(name="wmlp", bufs=1) as mwpool:
                    h1 = mpool.tile([P, FK, SP], FP8, tag="h1")
                    h2 = mpool.tile([P, FK, SP], FP8, tag="h2")
                    stage_fc1(a1, a2, h1, h2, mpool, mwpool)
                    stage_fc2(h1, h2, x2, y_sb, mpool, mwpool)

                for ti, (t0, pt) in enumerate(TT):
                    rp = min(pt, S - t0)
                    nc.sync.dma_start(y_d[b, t0:t0 + rp, :], y_sb[:rp, ti])

    nc.compile()
    return nc


def _get_nc():
    global _NC_CACHE
    if _NC_CACHE is None:
        _NC_CACHE = _build()
    return _NC_CACHE


def kernel(**inputs):
    nc = _get_nc()
    shared = prepare_shared(inputs)
    x = np.asarray(inputs["x"], np.float32).astype(BFNP)
    in_maps = []
    for i in range(NCORES):
        m = dict(shared)
        m["x"] = np.ascontiguousarray(x[i * BL:(i + 1) * BL])
        in_maps.append(m)
    res = bass_utils.run_bass_kernel_spmd(nc, in_maps,
                                          core_ids=list(range(NCORES)))
    y = np.concatenate([np.asarray(res.results[i]["y"])
                        for i in range(NCORES)], axis=0)
    return y.astype(np.float32)


# revision 10
# speedup vs baseline: 1.2225x; 1.0421x over previous
"""Trainium2 Bass kernel for a dense transformer block (pre-LN attention +
GELU MLP) — fp8e4m3 DoubleRow edition.

Strategy: data-parallel over batch across 8 NeuronCores (2 batches/core, no
collectives).  All matmuls run in fp8e4m3 with MatmulPerfMode.DoubleRow
(2 k-tiles per instruction at 0.5 cycles/row = 4x the fp32r rate).  Accuracy
is held by:
  - weights pre-scaled by 64/128 into fp8's normal range (inverse scale is
    folded into the free scalar slots of psum->SBUF copy ops),
  - residual-split operands: the MLP input and hidden activations are
    represented as main+residual fp8 pairs (a1+a2, h1+h2), and w1/w2 carry a
    matched-scale fp8 residual term (w1b/w2b), so the MLP is computed to
    ~0.3% while still running entirely at DoubleRow rate,
  - softmax without max-subtraction: es = exp(s - 3.2) stored in fp8 (the
    constant bias cancels in the normalization; max score ~8.2 so es < 240).
LayerNorm beta terms are folded into the following layer's biases on the host
(exactly linear), so the on-chip LN applies only the gain.

Engine-overhead notes: psum accumulation groups span disjoint column
sub-chunks of one bank (start zeroes the whole 2KB region), so ACT/DVE
psum->SBUF ops cover 512-col blocks; the 8 per-tile LN transposes land in one
bank (8 x 128 bf16 = 2KB) and are applied with a single DVE op; gelu runs one
ACT pass to bf16 with the fp8 split produced on the (otherwise idle) GpSimd.
Streamed weights are pre-blocked on the host so every DMA descriptor is a
contiguous 4-8KB per-partition run.
"""

import numpy as np
import ml_dtypes

import concourse.bass as bass
import concourse.mybir as mybir
import concourse.tile as tile
from concourse import bacc, bass_utils
from concourse.masks import make_identity

# Problem shape (hardcoded per spec nn_Block_58652073394865)
B, S, D, H, F = 16, 577, 1024, 16, 4096
DH = D // H
NCORES = 8
BL = B // NCORES        # batches per core
P = 128
KK = D // P             # 8 chunks of the model dim
FK = F // P             # 32 chunks of the mlp dim
EPS = 1e-6

SP = 578                # tokens padded with one zero token
TT = [(0, 128), (128, 128), (256, 128), (384, 128), (512, 66)]
# psum groups: [0,512) as two 256-wide DR sub-chunks in one bank, then [512,578)
CH = [(0, 512, (256, 256)), (512, 66, (66,))]
SC = [(0, 512), (512, 66)]               # plain-fp8 score chunks (out <= 512)
VS = 66                 # per-head stride in v (64 v + 1 ones + 1 pad)

WSC = 64.0              # fp8 pre-scale for wq/wk/wv/wo/w1
W2SC = 128.0            # fp8 pre-scale for w2
CTXSC = 16.0            # fp8 pre-scale for ctx
EXPB = 3.2              # softmax exp bias (cancels in normalization)
FC1_TERMS = 3           # 2: a1@w1a + a2@w1a;  3: + a1@w1b
FC2_TERMS = 3           # 2: h1@w2a + h2@w2a;  3: + h1@w2b

F32 = mybir.dt.float32
BF16 = mybir.dt.bfloat16
FP8 = mybir.dt.float8e4
U32 = mybir.dt.uint32
AF = mybir.ActivationFunctionType
OP = mybir.AluOpType
DR = mybir.MatmulPerfMode.DoubleRow

E4NP = ml_dtypes.float8_e4m3
BFNP = ml_dtypes.bfloat16

_NC_CACHE = None
# CoreSim doesn't implement the Gelu LUT; tests may swap this for AF.Tanh
_GELU = AF.Gelu

SHARED_NAMES = ["wq", "wk", "wv", "wo", "w1a", "w2a", "bq", "bk", "bv", "bo",
                "b1", "b2"]
if FC1_TERMS == 3:
    SHARED_NAMES.append("w1b")
if FC2_TERMS == 3:
    SHARED_NAMES.append("w2b")


def _block_layout(w8, nblk, cols):
    """[K, N] fp8 -> [nblk, 128, (K//128) * cols] with per-partition
    contiguous (ko, col) runs, matching SBUF tiles [P, K//128, cols]."""
    Kd, Nd = w8.shape
    ko = Kd // P
    # arr[b, p, k, c] = w8[k*128 + p, b*cols + c]
    a = w8.reshape(ko, P, nblk, cols).transpose(2, 1, 0, 3)
    return np.ascontiguousarray(a.reshape(nblk, P, ko * cols))


def prepare_shared(inputs):
    """Host-side: quantize/scale weights, fold LN betas into biases."""
    f = {n: np.ascontiguousarray(np.asarray(inputs[n], np.float32))
         for n in ("wq", "wk", "wv", "wo", "w1", "w2", "bq", "bk", "bv", "bo",
                   "b1", "b2", "ln1_g", "ln1_b", "ln2_g", "ln2_b")}

    def q8s(w, s):
        return (w * s).astype(E4NP)

    g1 = f["ln1_g"][:, None]
    g2 = f["ln2_g"][:, None]
    out = {
        "wq": _block_layout(q8s(g1 * f["wq"], WSC), 2, 512),
        "wk": _block_layout(q8s(g1 * f["wk"], WSC), 2, 512),
        "wv": _block_layout(q8s(g1 * f["wv"], WSC), 2, 512),
        "wo": _block_layout(q8s(f["wo"], WSC), 2, 512),
        "bq": f["bq"] + f["ln1_b"] @ f["wq"],
        "bk": f["bk"] + f["ln1_b"] @ f["wk"],
        "bv": f["bv"] + f["ln1_b"] @ f["wv"],
        "bo": np.ascontiguousarray(
            (f["bo"] * (WSC * CTXSC)).astype(BFNP)),
        "b1": f["b1"] + f["ln2_b"] @ f["w1"],
        "b2": f["b2"],
    }
    w1s = (g2 * f["w1"]) * WSC
    w1a = w1s.astype(E4NP)
    out["w1a"] = _block_layout(w1a, 8, 512)
    if FC1_TERMS == 3:
        out["w1b"] = _block_layout(
            (w1s - w1a.astype(np.float32)).astype(E4NP), 8, 512)
    w2s = f["w2"] * W2SC
    w2a = w2s.astype(E4NP)
    out["w2a"] = _block_layout(w2a, 4, 256)
    if FC2_TERMS == 3:
        out["w2b"] = _block_layout(
            (w2s - w2a.astype(np.float32)).astype(E4NP), 4, 256)
    return out


def _build():
    nc = bacc.Bacc("TRN2", target_bir_lowering=False, debug=False,
                   num_devices=NCORES)

    x_d = nc.dram_tensor("x", [BL, S, D], BF16, kind="ExternalInput").ap()
    y_d = nc.dram_tensor("y", [BL, S, D], BF16, kind="ExternalOutput").ap()
    wq_d = nc.dram_tensor("wq", [2, P, KK * 512], FP8, kind="ExternalInput").ap()
    wk_d = nc.dram_tensor("wk", [2, P, KK * 512], FP8, kind="ExternalInput").ap()
    wv_d = nc.dram_tensor("wv", [2, P, KK * 512], FP8, kind="ExternalInput").ap()
    wo_d = nc.dram_tensor("wo", [2, P, KK * 512], FP8, kind="ExternalInput").ap()
    w1a_d = nc.dram_tensor("w1a", [8, P, KK * 512], FP8, kind="ExternalInput").ap()
    w2a_d = nc.dram_tensor("w2a", [4, P, FK * 256], FP8, kind="ExternalInput").ap()
    w1b_d = (nc.dram_tensor("w1b", [8, P, KK * 512], FP8,
                            kind="ExternalInput").ap()
             if FC1_TERMS == 3 else None)
    w2b_d = (nc.dram_tensor("w2b", [4, P, FK * 256], FP8,
                            kind="ExternalInput").ap()
             if FC2_TERMS == 3 else None)
    bq_d = nc.dram_tensor("bq", [D], F32, kind="ExternalInput").ap()
    bk_d = nc.dram_tensor("bk", [D], F32, kind="ExternalInput").ap()
    bv_d = nc.dram_tensor("bv", [D], F32, kind="ExternalInput").ap()
    bo_d = nc.dram_tensor("bo", [D], BF16, kind="ExternalInput").ap()
    b1_d = nc.dram_tensor("b1", [F], F32, kind="ExternalInput").ap()
    b2_d = nc.dram_tensor("b2", [D], F32, kind="ExternalInput").ap()

    with tile.TileContext(nc) as tc:
        with tc.tile_pool(name="const", bufs=1) as cpool, \
             tc.tile_pool(name="resid", bufs=1) as rpool, \
             tc.tile_pool(name="fmbuf", bufs=1) as fmpool, \
             tc.tile_pool(name="lnp", bufs=2) as lnpool, \
             tc.tile_pool(name="ystg", bufs=1) as ypool, \
             tc.tile_pool(name="psA", bufs=4, space="PSUM") as psA:

            # ---- small params, packed ----
            cA = cpool.tile([P, 3 * KK + FK], F32, tag="cA")
            bq_sb = cA[:, 0:KK]
            bk_sb = cA[:, KK:2 * KK]
            b2_sb = cA[:, 2 * KK:3 * KK]
            b1_sb = cA[:, 3 * KK:3 * KK + FK]
            nc.scalar.dma_start(bq_sb, bq_d.rearrange("(m p) -> p m", p=P))
            nc.scalar.dma_start(bk_sb, bk_d.rearrange("(m p) -> p m", p=P))
            nc.scalar.dma_start(b2_sb, b2_d.rearrange("(m p) -> p m", p=P))
            nc.scalar.dma_start(b1_sb, b1_d.rearrange("(m p) -> p m", p=P))

            cB = cpool.tile([P, P + 3], F32, tag="cB")
            ident_f = cB[:, 0:P]
            epsap = cB[:, P:P + 1]
            onef = cB[:, P + 1:P + 2]
            nexpb = cB[:, P + 2:P + 3]
            make_identity(nc, ident_f)
            nc.vector.memset(epsap, EPS)
            nc.vector.memset(onef, 1.0)
            nc.vector.memset(nexpb, -EXPB)

            cC = cpool.tile([P, P + 4], BF16, tag="cC")
            ident_bf = cC[:, 0:P]
            nc.vector.tensor_copy(ident_bf, ident_f)
            cD = cpool.tile([1, P + D], BF16, tag="cD")
            ones_bf = cD[:, 0:P]
            bo_sb = cD[:, P:P + D]
            nc.vector.memset(ones_bf, 1.0)
            nc.scalar.dma_start(bo_sb, bo_d[None, :])

            onec8 = cpool.tile([P, 4], FP8, tag="onec8")
            nc.vector.tensor_copy(onec8[:, 0:1], onef)

            # bv broadcast to all partitions (bias varies along free dim)
            bvrow = cpool.tile([1, D], F32, tag="bvrow")
            nc.scalar.dma_start(bvrow[:], bv_d[None, :])
            bvb = cpool.tile([P, D], F32, tag="bvb")
            nc.gpsimd.partition_broadcast(bvb[:], bvrow[:])

            # ---- LayerNorm helpers (token-major stats; gain-only apply) ----
            def ln_new_stats():
                stats = lnpool.tile([P, 20], F32, tag="stats")
                nc.vector.memset(stats[:, 0:5], 0.0)
                nc.vector.memset(stats[:, 5:10], 1.0)
                return stats

            def ln_tile_stats(stats, src, ti, pt):
                negmu = stats[:, 0:5]
                varD = stats[:, 5:10]
                nc.vector.tensor_reduce(
                    negmu[:pt, ti:ti + 1], src[:pt, ti],
                    mybir.AxisListType.X, OP.add)
                nc.vector.tensor_scalar_mul(
                    negmu[:pt, ti:ti + 1], negmu[:pt, ti:ti + 1], -1.0 / D)
                scr = lnpool.tile([P, D], BF16, tag="xsq", bufs=2)
                nc.scalar.activation(
                    scr[:pt], src[:pt, ti], AF.Square,
                    bias=negmu[:pt, ti:ti + 1], accum_out=varD[:pt, ti:ti + 1])

            def ln_finalize(stats, lo, hi):
                nc.scalar.activation(stats[:, 10 + lo:10 + hi],
                                     stats[:, 5 + lo:5 + hi], AF.Sqrt,
                                     scale=1.0 / D, bias=epsap[:])
                nc.vector.reciprocal(stats[:, 15 + lo:15 + hi],
                                     stats[:, 10 + lo:10 + hi])

            def ln_apply_tiles(stats, src, dst1, dst2, tis):
                # normalize (bf16), 8 PE-transposes into ONE psum bank, then
                # plain fp8 copies (LN gains are folded into the weights).
                # dst2 (if not None) receives the fp8 residual split.
                negmu = stats[:, 0:5]
                rsig = stats[:, 15:20]
                for ti in tis:
                    t0, pt = TT[ti]
                    xn = lnpool.tile([P, D], BF16, tag="xn_tm", bufs=2)
                    nc.vector.tensor_scalar(
                        xn[:pt], src[:pt, ti],
                        negmu[:pt, ti:ti + 1], rsig[:pt, ti:ti + 1],
                        OP.add, OP.mult)
                    pst = psA.tile([P, 512], F32, tag="pA")
                    pbf = pst[:].bitcast(BF16).rearrange(
                        "p (k c) -> p k c", c=P)
                    for kk in range(KK):
                        nc.tensor.matmul(
                            pbf[:, kk, :pt],
                            xn[:pt, kk * P:(kk + 1) * P],
                            ident_bf[:pt, :pt],
                            is_transpose=True,
                            start=(kk == 0), stop=(kk == KK - 1))
                    nc.scalar.copy(dst1[:, :, t0:t0 + pt], pbf[:, :, :pt])
                    if dst2 is not None:
                        nc.vector.tensor_tensor(
                            dst2[:, :, t0:t0 + pt], pbf[:, :, :pt],
                            dst1[:, :, t0:t0 + pt], OP.subtract)

            def load_x(xb, b):
                nc.vector.memset(xb[64:, 4, :].bitcast(U32), 0)
                for ti, (t0, pt) in enumerate(TT):
                    rp = min(pt, S - t0)
                    nc.sync.dma_start(xb[:rp, ti], x_d[b, t0:t0 + rp, :])

            # ================= per-batch stages =================

            def stage_qkv(apool, xn1_fm, q8, k8, v_sb):
                # Q/K: feature-major out; V: token-major out (+ones col).
                nc.vector.memset(v_sb[:].bitcast(U32), 0)
                v_hc = v_sb[:].rearrange("p t (h c) -> p t h c", c=VS)
                nc.vector.tensor_copy(
                    v_hc[:, 0:4, :, 64:65],
                    onec8[:, 0:1, None, None].to_broadcast((P, 4, H, 1)))
                nc.vector.tensor_copy(
                    v_hc[:65, 4:5, :, 64:65],
                    onec8[:65, 0:1, None, None].to_broadcast((65, 1, H, 1)))

                for w_d, bias_sb, dst, use_act in ((wq_d, bq_sb, q8, True),
                                                   (wk_d, bk_sb, k8, False)):
                    for blk in range(2):
                        wt = apool.tile([P, KK, 512], FP8, tag="wqkv",
                                        bufs=3, name="wqkv")
                        nc.sync.dma_start(
                            wt[:], w_d[blk].rearrange("p (k c) -> p k c",
                                                      c=512))
                        for mi in range(4):
                            m = blk * 4 + mi
                            for (c0, cw, subs) in CH:
                                ps = psA.tile([P, 512], F32, tag="pA")
                                nsub = len(subs)
                                for si in range(nsub):
                                    s0 = c0 + si * 256
                                    sn = subs[si]
                                    for kp in range(4):
                                        nc.tensor.matmul(
                                            ps[:, si * 256:si * 256 + sn],
                                            wt[:, 2 * kp:2 * kp + 2,
                                               mi * P:(mi + 1) * P],
                                            xn1_fm[:, 2 * kp:2 * kp + 2,
                                                   s0:s0 + sn],
                                            start=(si == 0 and kp == 0),
                                            stop=(si == nsub - 1 and kp == 3),
                                            perf_mode=DR)
                                if use_act:
                                    nc.scalar.activation(
                                        dst[:, m, c0:c0 + cw], ps[:, :cw],
                                        AF.Identity,
                                        bias=bias_sb[:, m:m + 1],
                                        scale=1.0 / WSC)
                                else:
                                    nc.vector.tensor_scalar(
                                        dst[:, m, c0:c0 + cw], ps[:, :cw],
                                        1.0 / WSC, bias_sb[:, m:m + 1],
                                        OP.mult, OP.add)

                for blk in range(2):
                    wt = apool.tile([P, KK, 512], FP8, tag="wqkv",
                                    bufs=3, name="wqkv")
                    nc.sync.dma_start(
                        wt[:], wv_d[blk].rearrange("p (k c) -> p k c", c=512))
                    for ti, (t0, pt) in enumerate(TT):
                        rp = min(pt, S - t0)
                        ps = psA.tile([P, 512], F32, tag="pA")
                        for cc in range(2):
                            for kp in range(4):
                                nc.tensor.matmul(
                                    ps[:pt, cc * 256:(cc + 1) * 256],
                                    xn1_fm[:, 2 * kp:2 * kp + 2, t0:t0 + pt],
                                    wt[:, 2 * kp:2 * kp + 2,
                                       cc * 256:(cc + 1) * 256],
                                    start=(cc == 0 and kp == 0),
                                    stop=(cc == 1 and kp == 3),
                                    perf_mode=DR)
                        c0 = blk * 512
                        nc.vector.scalar_tensor_tensor(
                            v_hc[:rp, ti, 8 * blk:8 * (blk + 1), 0:64],
                            ps[:rp, :512].rearrange("p (h c) -> p h c", c=64),
                            1.0 / WSC,
                            bvb[:rp, c0:c0 + 512].rearrange(
                                "p (h c) -> p h c", c=64),
                            OP.mult, OP.add)

            def emit_scores(h, q8, k8, es):
                # es[:, kt, q] = exp(q.k/8 - EXPB) in fp8
                hrow = (h % 2) * 64
                kkh = h // 2
                for (c0, cn) in SC:
                    for pair in ((0, 1), (2, 3), (4,)):
                        pg = psA.tile([P, 2, 512], F32, tag="pS", bufs=2,
                                      name="pg")
                        for j, kt in enumerate(pair):
                            t0, ptk = TT[kt]
                            nc.tensor.matmul(
                                pg[:ptk, j, :cn],
                                k8[hrow:hrow + 64, kkh, t0:t0 + ptk],
                                q8[hrow:hrow + 64, kkh, c0:c0 + cn],
                                start=True, stop=True)
                        npair = len(pair)
                        prow = TT[pair[0]][1]
                        nc.scalar.activation(
                            es[:prow, pair[0]:pair[0] + npair, c0:c0 + cn],
                            pg[:prow, :npair, :cn],
                            AF.Exp, scale=1.0 / np.sqrt(DH),
                            bias=nexpb[:prow])

            def emit_pv(h, es, v_sb, ctx_fm):
                hrow = (h % 2) * 64
                kkh = h // 2
                for (c0, cw, subs) in CH:
                    pc = psA.tile([P, 512], F32, tag="pA")
                    nsub = len(subs)
                    for si in range(nsub):
                        s0 = c0 + si * 256
                        sn = subs[si]
                        for kp in range(2):
                            nc.tensor.matmul(
                                pc[:VS, si * 256:si * 256 + sn],
                                v_sb[:, 2 * kp:2 * kp + 2,
                                     h * VS:(h + 1) * VS],
                                es[:, 2 * kp:2 * kp + 2, s0:s0 + sn],
                                start=(si == 0 and kp == 0), stop=False,
                                perf_mode=DR)
                        nc.tensor.matmul(
                            pc[:VS, si * 256:si * 256 + sn],
                            v_sb[:66, 4, h * VS:(h + 1) * VS],
                            es[:66, 4, s0:s0 + sn],
                            start=False, stop=(si == nsub - 1))
                    rc = lnpool.tile([1, 512], BF16, tag="rc", bufs=2)
                    with nc.allow_low_precision(reason="softmax 1/Z bf16"):
                        nc.vector.reciprocal(rc[:, :cw], pc[64:65, :cw])
                    rb = lnpool.tile([64, 512], BF16, tag="rb", bufs=2)
                    nc.gpsimd.partition_broadcast(rb[:, :cw], rc[:, :cw])
                    nc.vector.scalar_tensor_tensor(
                        ctx_fm[hrow:hrow + 64, kkh, c0:c0 + cw],
                        pc[0:64, :cw], CTXSC, rb[:, :cw],
                        OP.mult, OP.mult)

            def stage_oproj(apool, ctx_fm, xb, x2, stats2):
                # out token-major: x2 = attn/(WSC*CTXSC) + x ; LN2 stats after
                for blk in range(2):
                    wt = apool.tile([P, KK, 512], FP8, tag="wqkv",
                                    bufs=3, name="wqkv")
                    nc.sync.dma_start(
                        wt[:], wo_d[blk].rearrange("p (k c) -> p k c", c=512))
                    c0 = blk * 512
                    for ti in (4, 0, 1, 2, 3):
                        t0, pt = TT[ti]
                        ps = psA.tile([P, 512], F32, tag="pA")
                        for cc in range(2):
                            for kp in range(4):
                                nc.tensor.matmul(
                                    ps[:pt, cc * 256:(cc + 1) * 256],
                                    ctx_fm[:, 2 * kp:2 * kp + 2, t0:t0 + pt],
                                    wt[:, 2 * kp:2 * kp + 2,
                                       cc * 256:(cc + 1) * 256],
                                    start=(cc == 0 and kp == 0), stop=False,
                                    perf_mode=DR)
                        nc.tensor.matmul(
                            ps[:pt, :512], ones_bf[:1, :pt],
                            bo_sb[:1, c0:c0 + 512], start=False, stop=True)
                        nc.vector.scalar_tensor_tensor(
                            x2[:pt, ti, c0:c0 + 512], ps[:pt, :512],
                            1.0 / (WSC * CTXSC), xb[:pt, ti, c0:c0 + 512],
                            OP.mult, OP.add)
                for ti, (t0, pt) in enumerate(TT):
                    ln_tile_stats(stats2, x2, ti, pt)

            def stage_fc1(a1, a2, h1, h2, mpool, mwpool):
                for blk in range(8):
                    wa = mwpool.tile([P, KK, 512], FP8, tag="w1a", bufs=3)
                    nc.sync.dma_start(
                        wa[:], w1a_d[blk].rearrange("p (k c) -> p k c",
                                                    c=512))
                    if FC1_TERMS == 3:
                        wb = mwpool.tile([P, KK, 512], FP8, tag="w1b",
                                         bufs=3)
                        nc.sync.dma_start(
                            wb[:], w1b_d[blk].rearrange("p (k c) -> p k c",
                                                        c=512))
                    for mi in range(4):
                        m = blk * 4 + mi
                        mc = slice(mi * P, (mi + 1) * P)
                        hb = mpool.tile([P, SP], BF16, tag="hb", bufs=2)
                        for (c0, cw, subs) in CH:
                            ps = psA.tile([P, 512], F32, tag="pA")
                            terms = [(a1, wa), (a2, wa)]
                            if FC1_TERMS == 3:
                                terms.append((a1, wb))
                            nterm = len(terms)
                            nsub = len(subs)
                            for si in range(nsub):
                                s0 = c0 + si * 256
                                sn = subs[si]
                                for tix, (asrc, wsrc) in enumerate(terms):
                                    for kp in range(4):
                                        nc.tensor.matmul(
                                            ps[:, si * 256:si * 256 + sn],
                                            wsrc[:, 2 * kp:2 * kp + 2, mc],
                                            asrc[:, 2 * kp:2 * kp + 2,
                                                 s0:s0 + sn],
                                            start=(si == 0 and tix == 0
                                                   and kp == 0),
                                            stop=(si == nsub - 1 and
                                                  tix == nterm - 1 and
                                                  kp == 3),
                                            perf_mode=DR)
                            nc.scalar.activation(
                                hb[:, c0:c0 + cw], ps[:, :cw], _GELU,
                                bias=b1_sb[:, m:m + 1], scale=1.0 / WSC)
                        nc.gpsimd.tensor_copy(h1[:, m, :], hb[:, :])
                        nc.gpsimd.tensor_tensor(
                            h2[:, m, :], hb[:, :], h1[:, m, :], OP.subtract)

            def stage_fc2(h1, h2, x2, y_sb, mpool, mwpool):
                for blk in range(4):
                    wa = mwpool.tile([P, FK, 256], FP8, tag="w2a", bufs=3)
                    nc.sync.dma_start(
                        wa[:], w2a_d[blk].rearrange("p (k c) -> p k c",
                                                    c=256))
                    if FC2_TERMS == 3:
                        wb = mwpool.tile([P, FK, 256], FP8, tag="w2b",
                                         bufs=3)
                        nc.sync.dma_start(
                            wb[:], w2b_d[blk].rearrange("p (k c) -> p k c",
                                                        c=256))
                    for mi in range(2):
                        m = blk * 2 + mi
                        mc = slice(mi * P, (mi + 1) * P)
                        mlp_fm = mpool.tile([P, SP], BF16, tag="mlp_fm",
                                            bufs=2)
                        for (c0, cw, subs) in CH:
                            ps = psA.tile([P, 512], F32, tag="pA")
                            terms = [(h1, wa), (h2, wa)]
                            if FC2_TERMS == 3:
                                terms.append((h1, wb))
                            nterm = len(terms)
                            nsub = len(subs)
                            for si in range(nsub):
                                s0 = c0 + si * 256
                                sn = subs[si]
                                for tix, (hsrc, wsrc) in enumerate(terms):
                                    for kp in range(FK // 2):
                                        nc.tensor.matmul(
                                            ps[:, si * 256:si * 256 + sn],
                                            wsrc[:, 2 * kp:2 * kp + 2, mc],
                                            hsrc[:, 2 * kp:2 * kp + 2,
                                                 s0:s0 + sn],
                                            start=(si == 0 and tix == 0
                                                   and kp == 0),
                                            stop=(si == nsub - 1 and
                                                  tix == nterm - 1 and
                                                  kp == FK // 2 - 1),
                                            perf_mode=DR)
                            nc.vector.tensor_scalar(
                                mlp_fm[:, c0:c0 + cw], ps[:, :cw],
                                1.0 / W2SC, b2_sb[:, m:m + 1],
                                OP.mult, OP.add)
                        # 5 transposes into one bank; single residual op.
                        # (tile0's start zeroes the whole region, so the
                        # garbage rows of tile4 read as zeros.)
                        pst = psA.tile([P, 512], F32, tag="pA")
                        pbf = pst[:].bitcast(BF16).rearrange(
                            "p (t c) -> p t c", c=P)
                        for ti, (t0, pt) in enumerate(TT):
                            nc.tensor.matmul(
                                pbf[:pt, ti, :P], mlp_fm[:, t0:t0 + pt],
                                ident_bf[:], is_transpose=True,
                                start=(ti == 0), stop=(ti == 4))
                        nc.vector.scalar_tensor_tensor(
                            y_sb[:, :, m * P:(m + 1) * P],
                            pbf[:, 0:5, :], 0.0,
                            x2[:, :, m * P:(m + 1) * P],
                            OP.add, OP.add)

            # ================= batch loop =================
            for b in range(BL):
                xb = rpool.tile([P, 5, D], BF16, tag="xb", bufs=2)
                load_x(xb, b)
                stats1 = ln_new_stats()
                for ti, (t0, pt) in enumerate(TT):
                    ln_tile_stats(stats1, xb, ti, pt)

                xn1_fm = fmpool.tile([P, KK, SP], FP8, tag="xn1_fm")
                ln_finalize(stats1, 0, 2)
                ln_apply_tiles(stats1, xb, xn1_fm, None, (0, 1))
                ln_finalize(stats1, 2, 5)
                ln_apply_tiles(stats1, xb, xn1_fm, None, (2, 3, 4))

                x2 = rpool.tile([P, 5, D], BF16, tag="x2", bufs=1)
                with tc.tile_pool(name="attn", bufs=1) as apool:
                    q8 = apool.tile([P, KK, SP], FP8, tag="q8")
                    k8 = apool.tile([P, KK, SP], FP8, tag="k8")
                    v_sb = apool.tile([P, 5, H * VS], FP8, tag="v")
                    ctx_fm = apool.tile([P, KK, SP], FP8, tag="ctx")

                    stage_qkv(apool, xn1_fm, q8, k8, v_sb)

                    def get_es(i):
                        return apool.tile([P, 5, SP], FP8, tag=f"es{i % 2}",
                                          name=f"es{i % 2}")

                    es_cur = get_es(0)
                    emit_scores(0, q8, k8, es_cur)
                    for h in range(H):
                        if h + 1 < H:
                            es_nxt = get_es(h + 1)
                            emit_scores(h + 1, q8, k8, es_nxt)
                        emit_pv(h, es_cur, v_sb, ctx_fm)
                        if h + 1 < H:
                            es_cur = es_nxt

                    stats2 = ln_new_stats()
                    stage_oproj(apool, ctx_fm, xb, x2, stats2)

                # ---- LN2 -> a1 (+a2 residual split) ----
                a1 = fmpool.tile([P, KK, SP], FP8, tag="a1")
                a2 = fmpool.tile([P, KK, SP], FP8, tag="a2")
                ln_finalize(stats2, 0, 5)
                ln_apply_tiles(stats2, x2, a1, a2, (0, 1, 2, 3, 4))

                y_sb = ypool.tile([P, 5, D], BF16, tag="y_sb")
                with tc.tile_pool(name="mlp", bufs=1) as mpool, \
                     tc.tile_pool(name="wmlp", bufs=1) as mwpool:
                    h1 = mpool.tile([P, FK, SP], FP8, tag="h1")
                    h2 = mpool.tile([P, FK, SP], FP8, tag="h2")
                    stage_fc1(a1, a2, h1, h2, mpool, mwpool)
                    stage_fc2(h1, h2, x2, y_sb, mpool, mwpool)

                for ti, (t0, pt) in enumerate(TT):
                    rp = min(pt, S - t0)
                    nc.sync.dma_start(y_d[b, t0:t0 + rp, :], y_sb[:rp, ti])

    nc.compile()
    return nc


def _get_nc():
    global _NC_CACHE
    if _NC_CACHE is None:
        _NC_CACHE = _build()
    return _NC_CACHE


def kernel(**inputs):
    nc = _get_nc()
    shared = prepare_shared(inputs)
    x = np.asarray(inputs["x"], np.float32).astype(BFNP)
    in_maps = []
    for i in range(NCORES):
        m = dict(shared)
        m["x"] = np.ascontiguousarray(x[i * BL:(i + 1) * BL])
        in_maps.append(m)
    res = bass_utils.run_bass_kernel_spmd(nc, in_maps,
                                          core_ids=list(range(NCORES)))
    y = np.concatenate([np.asarray(res.results[i]["y"])
                        for i in range(NCORES)], axis=0)
    return y.astype(np.float32)


# revision 12
# speedup vs baseline: 1.2500x; 1.0225x over previous
"""Trainium2 Bass kernel for a dense transformer block (pre-LN attention +
GELU MLP) — fp8e4m3 DoubleRow edition.

Strategy: data-parallel over batch across 8 NeuronCores (2 batches/core, no
collectives).  All matmuls run in fp8e4m3 with MatmulPerfMode.DoubleRow
(2 k-tiles per instruction at 0.5 cycles/row = 4x the fp32r rate).  Accuracy
is held by:
  - weights pre-scaled by 64/128 into fp8's normal range (inverse scale is
    folded into the free scalar slots of psum->SBUF copy ops),
  - residual-split operands: the MLP input and hidden activations are
    represented as main+residual fp8 pairs (a1+a2, h1+h2), and w1/w2 carry a
    matched-scale fp8 residual term (w1b/w2b), so the MLP is computed to
    ~0.3% while still running entirely at DoubleRow rate,
  - softmax without max-subtraction: es = exp(s - 3.2) stored in fp8 (the
    constant bias cancels in the normalization; max score ~8.2 so es < 240).
LayerNorm beta terms are folded into the following layer's biases on the host
(exactly linear), so the on-chip LN applies only the gain.

Engine-overhead notes: psum accumulation groups span disjoint column
sub-chunks of one bank (start zeroes the whole 2KB region), so ACT/DVE
psum->SBUF ops cover 512-col blocks; the 8 per-tile LN transposes land in one
bank (8 x 128 bf16 = 2KB) and are applied with a single DVE op; gelu runs one
ACT pass to bf16 with the fp8 split produced on the (otherwise idle) GpSimd.
Streamed weights are pre-blocked on the host so every DMA descriptor is a
contiguous 4-8KB per-partition run.
"""

import numpy as np
import ml_dtypes

import concourse.bass as bass
import concourse.mybir as mybir
import concourse.tile as tile
from concourse import bacc, bass_utils
from concourse.masks import make_identity

# Problem shape (hardcoded per spec nn_Block_58652073394865)
B, S, D, H, F = 16, 577, 1024, 16, 4096
DH = D // H
NCORES = 8
BL = B // NCORES        # batches per core
P = 128
KK = D // P             # 8 chunks of the model dim
FK = F // P             # 32 chunks of the mlp dim
EPS = 1e-6

SP = 578                # tokens padded with one zero token
TT = [(0, 128), (128, 128), (256, 128), (384, 128), (512, 66)]
# psum groups: [0,512) as two 256-wide DR sub-chunks in one bank, then [512,578)
CH = [(0, 512, (256, 256)), (512, 66, (66,))]
SC = [(0, 512), (512, 66)]               # plain-fp8 score chunks (out <= 512)
VS = 66                 # per-head stride in v (64 v + 1 ones + 1 pad)

WSC = 64.0              # fp8 pre-scale for wq/wk/wv/wo/w1
W2SC = 128.0            # fp8 pre-scale for w2
CTXSC = 16.0            # fp8 pre-scale for ctx
EXPB = 3.2              # softmax exp bias (cancels in normalization)
FC1_TERMS = 3           # 2: a1@w1a + a2@w1a;  3: + a1@w1b
FC2_TERMS = 3           # 2: h1@w2a + h2@w2a;  3: + h1@w2b

F32 = mybir.dt.float32
BF16 = mybir.dt.bfloat16
FP8 = mybir.dt.float8e4
U32 = mybir.dt.uint32
AF = mybir.ActivationFunctionType
OP = mybir.AluOpType
DR = mybir.MatmulPerfMode.DoubleRow

E4NP = ml_dtypes.float8_e4m3
BFNP = ml_dtypes.bfloat16

_NC_CACHE = None
# CoreSim doesn't implement the Gelu LUT; tests may swap this for AF.Tanh
_GELU = AF.Gelu

SHARED_NAMES = ["wq", "wk", "wv", "wo", "w1a", "w2a", "bq", "bk", "bv", "bo",
                "b1", "b2"]
if FC1_TERMS == 3:
    SHARED_NAMES.append("w1b")
if FC2_TERMS == 3:
    SHARED_NAMES.append("w2b")


def _block_layout(w8, nblk, cols):
    """[K, N] fp8 -> [nblk, 128, (K//128) * cols] with per-partition
    contiguous (ko, col) runs, matching SBUF tiles [P, K//128, cols]."""
    Kd, Nd = w8.shape
    ko = Kd // P
    # arr[b, p, k, c] = w8[k*128 + p, b*cols + c]
    a = w8.reshape(ko, P, nblk, cols).transpose(2, 1, 0, 3)
    return np.ascontiguousarray(a.reshape(nblk, P, ko * cols))


def prepare_shared(inputs):
    """Host-side: quantize/scale weights, fold LN betas into biases."""
    f = {n: np.ascontiguousarray(np.asarray(inputs[n], np.float32))
         for n in ("wq", "wk", "wv", "wo", "w1", "w2", "bq", "bk", "bv", "bo",
                   "b1", "b2", "ln1_g", "ln1_b", "ln2_g", "ln2_b")}

    def q8s(w, s):
        return (w * s).astype(E4NP)

    g1 = f["ln1_g"][:, None]
    g2 = f["ln2_g"][:, None]
    out = {
        "wq": _block_layout(q8s(g1 * f["wq"], WSC), 2, 512),
        "wk": _block_layout(q8s(g1 * f["wk"], WSC), 2, 512),
        "wv": _block_layout(q8s(g1 * f["wv"], WSC), 2, 512),
        "wo": _block_layout(q8s(f["wo"], WSC), 2, 512),
        "bq": f["bq"] + f["ln1_b"] @ f["wq"],
        "bk": f["bk"] + f["ln1_b"] @ f["wk"],
        "bv": np.ascontiguousarray((f["bv"] + f["ln1_b"] @ f["wv"]).astype(BFNP)),
        "bo": np.ascontiguousarray(
            (f["bo"] * (WSC * CTXSC)).astype(BFNP)),
        "b1": f["b1"] + f["ln2_b"] @ f["w1"],
        "b2": f["b2"],
    }
    w1s = (g2 * f["w1"]) * WSC
    w1a = w1s.astype(E4NP)
    out["w1a"] = _block_layout(w1a, 8, 512)
    if FC1_TERMS == 3:
        out["w1b"] = _block_layout(
            (w1s - w1a.astype(np.float32)).astype(E4NP), 8, 512)
    w2s = f["w2"] * W2SC
    w2a = w2s.astype(E4NP)
    out["w2a"] = _block_layout(w2a, 8, 128)
    if FC2_TERMS == 3:
        out["w2b"] = _block_layout(
            (w2s - w2a.astype(np.float32)).astype(E4NP), 8, 128)
    return out


def _build():
    nc = bacc.Bacc("TRN2", target_bir_lowering=False, debug=False,
                   num_devices=NCORES)

    x_d = nc.dram_tensor("x", [BL, S, D], BF16, kind="ExternalInput").ap()
    y_d = nc.dram_tensor("y", [BL, S, D], BF16, kind="ExternalOutput").ap()
    wq_d = nc.dram_tensor("wq", [2, P, KK * 512], FP8, kind="ExternalInput").ap()
    wk_d = nc.dram_tensor("wk", [2, P, KK * 512], FP8, kind="ExternalInput").ap()
    wv_d = nc.dram_tensor("wv", [2, P, KK * 512], FP8, kind="ExternalInput").ap()
    wo_d = nc.dram_tensor("wo", [2, P, KK * 512], FP8, kind="ExternalInput").ap()
    w1a_d = nc.dram_tensor("w1a", [8, P, KK * 512], FP8, kind="ExternalInput").ap()
    w2a_d = nc.dram_tensor("w2a", [8, P, FK * 128], FP8, kind="ExternalInput").ap()
    w1b_d = (nc.dram_tensor("w1b", [8, P, KK * 512], FP8,
                            kind="ExternalInput").ap()
             if FC1_TERMS == 3 else None)
    w2b_d = (nc.dram_tensor("w2b", [8, P, FK * 128], FP8,
                            kind="ExternalInput").ap()
             if FC2_TERMS == 3 else None)
    bq_d = nc.dram_tensor("bq", [D], F32, kind="ExternalInput").ap()
    bk_d = nc.dram_tensor("bk", [D], F32, kind="ExternalInput").ap()
    bv_d = nc.dram_tensor("bv", [D], BF16, kind="ExternalInput").ap()
    bo_d = nc.dram_tensor("bo", [D], BF16, kind="ExternalInput").ap()
    b1_d = nc.dram_tensor("b1", [F], F32, kind="ExternalInput").ap()
    b2_d = nc.dram_tensor("b2", [D], F32, kind="ExternalInput").ap()

    with tile.TileContext(nc) as tc:
        with tc.tile_pool(name="const", bufs=1) as cpool, \
             tc.tile_pool(name="resid", bufs=1) as rpool, \
             tc.tile_pool(name="fmbuf", bufs=1) as fmpool, \
             tc.tile_pool(name="lnp", bufs=2) as lnpool, \
             tc.tile_pool(name="ystg", bufs=1) as ypool, \
             tc.tile_pool(name="psA", bufs=4, space="PSUM") as psA:

            # ---- small params, packed ----
            cA = cpool.tile([P, 3 * KK + FK], F32, tag="cA")
            bq_sb = cA[:, 0:KK]
            bk_sb = cA[:, KK:2 * KK]
            b2_sb = cA[:, 2 * KK:3 * KK]
            b1_sb = cA[:, 3 * KK:3 * KK + FK]
            nc.scalar.dma_start(bq_sb, bq_d.rearrange("(m p) -> p m", p=P))
            nc.scalar.dma_start(bk_sb, bk_d.rearrange("(m p) -> p m", p=P))
            nc.scalar.dma_start(b2_sb, b2_d.rearrange("(m p) -> p m", p=P))
            nc.scalar.dma_start(b1_sb, b1_d.rearrange("(m p) -> p m", p=P))

            cB = cpool.tile([P, P + 3], F32, tag="cB")
            ident_f = cB[:, 0:P]
            epsap = cB[:, P:P + 1]
            onef = cB[:, P + 1:P + 2]
            nexpb = cB[:, P + 2:P + 3]
            make_identity(nc, ident_f)
            nc.vector.memset(epsap, EPS)
            nc.vector.memset(onef, 1.0)
            nc.vector.memset(nexpb, -EXPB)

            cC = cpool.tile([P, P + 4], BF16, tag="cC")
            ident_bf = cC[:, 0:P]
            nc.vector.tensor_copy(ident_bf, ident_f)
            cD = cpool.tile([1, P + D], BF16, tag="cD")
            ones_bf = cD[:, 0:P]
            bo_sb = cD[:, P:P + D]
            nc.vector.memset(ones_bf, 1.0)
            nc.scalar.dma_start(bo_sb, bo_d[None, :])

            onec8 = cpool.tile([P, 4], FP8, tag="onec8")
            nc.vector.tensor_copy(onec8[:, 0:1], onef)

            # bv broadcast to all partitions (bias varies along free dim)
            bvrow = cpool.tile([1, D], BF16, tag="bvrow")
            nc.scalar.dma_start(bvrow[:], bv_d[None, :])
            bvb = cpool.tile([P, D], BF16, tag="bvb")
            nc.gpsimd.partition_broadcast(bvb[:], bvrow[:])

            # ---- LayerNorm helpers (token-major stats; gain-only apply) ----
            def ln_new_stats():
                stats = lnpool.tile([P, 20], F32, tag="stats")
                nc.vector.memset(stats[:, 0:5], 0.0)
                nc.vector.memset(stats[:, 5:10], 1.0)
                return stats

            def ln_tile_stats(stats, src, ti, pt):
                negmu = stats[:, 0:5]
                varD = stats[:, 5:10]
                nc.vector.tensor_reduce(
                    negmu[:pt, ti:ti + 1], src[:pt, ti],
                    mybir.AxisListType.X, OP.add)
                nc.vector.tensor_scalar_mul(
                    negmu[:pt, ti:ti + 1], negmu[:pt, ti:ti + 1], -1.0 / D)
                scr = lnpool.tile([P, D], BF16, tag="xsq", bufs=2)
                nc.scalar.activation(
                    scr[:pt], src[:pt, ti], AF.Square,
                    bias=negmu[:pt, ti:ti + 1], accum_out=varD[:pt, ti:ti + 1])

            def ln_finalize(stats, lo, hi):
                nc.scalar.activation(stats[:, 10 + lo:10 + hi],
                                     stats[:, 5 + lo:5 + hi], AF.Sqrt,
                                     scale=1.0 / D, bias=epsap[:])
                nc.vector.reciprocal(stats[:, 15 + lo:15 + hi],
                                     stats[:, 10 + lo:10 + hi])

            def ln_apply_tiles(stats, src, dst1, dst2, tis):
                # normalize (bf16), 8 PE-transposes into ONE psum bank, then
                # plain fp8 copies (LN gains are folded into the weights).
                # dst2 (if not None) receives the fp8 residual split.
                negmu = stats[:, 0:5]
                rsig = stats[:, 15:20]
                for ti in tis:
                    t0, pt = TT[ti]
                    xn = lnpool.tile([P, D], BF16, tag="xn_tm", bufs=2)
                    nc.vector.tensor_scalar(
                        xn[:pt], src[:pt, ti],
                        negmu[:pt, ti:ti + 1], rsig[:pt, ti:ti + 1],
                        OP.add, OP.mult)
                    pst = psA.tile([P, 512], F32, tag="pA")
                    pbf = pst[:].bitcast(BF16).rearrange(
                        "p (k c) -> p k c", c=P)
                    for kk in range(KK):
                        nc.tensor.matmul(
                            pbf[:, kk, :pt],
                            xn[:pt, kk * P:(kk + 1) * P],
                            ident_bf[:pt, :pt],
                            is_transpose=True,
                            start=(kk == 0), stop=(kk == KK - 1))
                    nc.scalar.copy(dst1[:, :, t0:t0 + pt], pbf[:, :, :pt])
                    if dst2 is not None:
                        nc.vector.tensor_tensor(
                            dst2[:, :, t0:t0 + pt], pbf[:, :, :pt],
                            dst1[:, :, t0:t0 + pt], OP.subtract)

            def load_x(xb, b):
                nc.vector.memset(xb[64:, 4, :].bitcast(U32), 0)
                for ti, (t0, pt) in enumerate(TT):
                    rp = min(pt, S - t0)
                    nc.sync.dma_start(xb[:rp, ti], x_d[b, t0:t0 + rp, :])

            # ================= per-batch stages =================

            def gen_qkv(apool, xn1_fm, q8, k8, v_sb):
                # Q/K: feature-major out; V: token-major out (+ones col).
                nc.vector.memset(v_sb[:].bitcast(U32), 0)
                v_hc = v_sb[:].rearrange("p t (h c) -> p t h c", c=VS)
                nc.vector.tensor_copy(
                    v_hc[:, 0:4, :, 64:65],
                    onec8[:, 0:1, None, None].to_broadcast((P, 4, H, 1)))
                nc.vector.tensor_copy(
                    v_hc[:65, 4:5, :, 64:65],
                    onec8[:65, 0:1, None, None].to_broadcast((65, 1, H, 1)))

                for w_d, bias_sb, dst, use_act in ((wq_d, bq_sb, q8, True),
                                                   (wk_d, bk_sb, k8, False)):
                    for blk in range(2):
                        yield
                        wt = apool.tile([P, KK, 512], FP8, tag="wqkv",
                                        bufs=3, name="wqkv")
                        nc.sync.dma_start(
                            wt[:], w_d[blk].rearrange("p (k c) -> p k c",
                                                      c=512))
                        for mi in range(4):
                            m = blk * 4 + mi
                            for (c0, cw, subs) in CH:
                                ps = psA.tile([P, 512], F32, tag="pA")
                                nsub = len(subs)
                                for si in range(nsub):
                                    s0 = c0 + si * 256
                                    sn = subs[si]
                                    for kp in range(4):
                                        nc.tensor.matmul(
                                            ps[:, si * 256:si * 256 + sn],
                                            wt[:, 2 * kp:2 * kp + 2,
                                               mi * P:(mi + 1) * P],
                                            xn1_fm[:, 2 * kp:2 * kp + 2,
                                                   s0:s0 + sn],
                                            start=(si == 0 and kp == 0),
                                            stop=(si == nsub - 1 and kp == 3),
                                            perf_mode=DR)
                                if use_act:
                                    nc.scalar.activation(
                                        dst[:, m, c0:c0 + cw], ps[:, :cw],
                                        AF.Identity,
                                        bias=bias_sb[:, m:m + 1],
                                        scale=1.0 / WSC)
                                else:
                                    nc.vector.tensor_scalar(
                                        dst[:, m, c0:c0 + cw], ps[:, :cw],
                                        1.0 / WSC, bias_sb[:, m:m + 1],
                                        OP.mult, OP.add)

                for blk in range(2):
                    yield
                    wt = apool.tile([P, KK, 512], FP8, tag="wqkv",
                                    bufs=3, name="wqkv")
                    nc.sync.dma_start(
                        wt[:], wv_d[blk].rearrange("p (k c) -> p k c", c=512))
                    for ti, (t0, pt) in enumerate(TT):
                        rp = min(pt, S - t0)
                        ps = psA.tile([P, 512], F32, tag="pA")
                        for cc in range(2):
                            for kp in range(4):
                                nc.tensor.matmul(
                                    ps[:pt, cc * 256:(cc + 1) * 256],
                                    xn1_fm[:, 2 * kp:2 * kp + 2, t0:t0 + pt],
                                    wt[:, 2 * kp:2 * kp + 2,
                                       cc * 256:(cc + 1) * 256],
                                    start=(cc == 0 and kp == 0),
                                    stop=(cc == 1 and kp == 3),
                                    perf_mode=DR)
                        c0 = blk * 512
                        nc.vector.scalar_tensor_tensor(
                            v_hc[:rp, ti, 8 * blk:8 * (blk + 1), 0:64],
                            ps[:rp, :512].rearrange("p (h c) -> p h c", c=64),
                            1.0 / WSC,
                            bvb[:rp, c0:c0 + 512].rearrange(
                                "p (h c) -> p h c", c=64),
                            OP.mult, OP.add)

            def emit_scores(h, q8, k8, es):
                # es[:, kt, q] = exp(q.k/8 - EXPB) in fp8
                hrow = (h % 2) * 64
                kkh = h // 2
                for (c0, cn) in SC:
                    for pair in ((0, 1), (2, 3), (4,)):
                        pg = psA.tile([P, 2, 512], F32, tag="pS", bufs=2,
                                      name="pg")
                        for j, kt in enumerate(pair):
                            t0, ptk = TT[kt]
                            nc.tensor.matmul(
                                pg[:ptk, j, :cn],
                                k8[hrow:hrow + 64, kkh, t0:t0 + ptk],
                                q8[hrow:hrow + 64, kkh, c0:c0 + cn],
                                start=True, stop=True)
                        npair = len(pair)
                        prow = TT[pair[0]][1]
                        nc.scalar.activation(
                            es[:prow, pair[0]:pair[0] + npair, c0:c0 + cn],
                            pg[:prow, :npair, :cn],
                            AF.Exp, scale=1.0 / np.sqrt(DH),
                            bias=nexpb[:prow])

            def emit_pv(h, es, v_sb, ctx_fm):
                hrow = (h % 2) * 64
                kkh = h // 2
                for (c0, cw, subs) in CH:
                    pc = psA.tile([P, 512], F32, tag="pA")
                    nsub = len(subs)
                    for si in range(nsub):
                        s0 = c0 + si * 256
                        sn = subs[si]
                        for kp in range(2):
                            nc.tensor.matmul(
                                pc[:VS, si * 256:si * 256 + sn],
                                v_sb[:, 2 * kp:2 * kp + 2,
                                     h * VS:(h + 1) * VS],
                                es[:, 2 * kp:2 * kp + 2, s0:s0 + sn],
                                start=(si == 0 and kp == 0), stop=False,
                                perf_mode=DR)
                        nc.tensor.matmul(
                            pc[:VS, si * 256:si * 256 + sn],
                            v_sb[:66, 4, h * VS:(h + 1) * VS],
                            es[:66, 4, s0:s0 + sn],
                            start=False, stop=(si == nsub - 1))
                    rc = lnpool.tile([1, 512], BF16, tag="rc", bufs=2)
                    with nc.allow_low_precision(reason="softmax 1/Z bf16"):
                        nc.vector.reciprocal(rc[:, :cw], pc[64:65, :cw])
                    rb = lnpool.tile([64, 512], BF16, tag="rb", bufs=2)
                    nc.gpsimd.partition_broadcast(rb[:, :cw], rc[:, :cw])
                    nc.vector.scalar_tensor_tensor(
                        ctx_fm[hrow:hrow + 64, kkh, c0:c0 + cw],
                        pc[0:64, :cw], CTXSC, rb[:, :cw],
                        OP.mult, OP.mult)

            def stage_oproj(apool, ctx_fm, xb, x2, stats2):
                # out token-major: x2 = attn/(WSC*CTXSC) + x ; LN2 stats after
                for blk in range(2):
                    wt = apool.tile([P, KK, 512], FP8, tag="wqkv",
                                    bufs=3, name="wqkv")
                    nc.sync.dma_start(
                        wt[:], wo_d[blk].rearrange("p (k c) -> p k c", c=512))
                    c0 = blk * 512
                    for ti in (4, 0, 1, 2, 3):
                        t0, pt = TT[ti]
                        ps = psA.tile([P, 512], F32, tag="pA")
                        for cc in range(2):
                            for kp in range(4):
                                nc.tensor.matmul(
                                    ps[:pt, cc * 256:(cc + 1) * 256],
                                    ctx_fm[:, 2 * kp:2 * kp + 2, t0:t0 + pt],
                                    wt[:, 2 * kp:2 * kp + 2,
                                       cc * 256:(cc + 1) * 256],
                                    start=(cc == 0 and kp == 0), stop=False,
                                    perf_mode=DR)
                        nc.tensor.matmul(
                            ps[:pt, :512], ones_bf[:1, :pt],
                            bo_sb[:1, c0:c0 + 512], start=False, stop=True)
                        nc.vector.scalar_tensor_tensor(
                            x2[:pt, ti, c0:c0 + 512], ps[:pt, :512],
                            1.0 / (WSC * CTXSC), xb[:pt, ti, c0:c0 + 512],
                            OP.mult, OP.add)
                for ti, (t0, pt) in enumerate(TT):
                    ln_tile_stats(stats2, x2, ti, pt)

            def gen_fc1(a1, a2, h1, h2, mpool, mwpool):
                for blk in range(8):
                    yield
                    wa = mwpool.tile([P, KK, 512], FP8, tag="w1a", bufs=2)
                    nc.sync.dma_start(
                        wa[:], w1a_d[blk].rearrange("p (k c) -> p k c",
                                                    c=512))
                    if FC1_TERMS == 3:
                        wb = mwpool.tile([P, KK, 512], FP8, tag="w1b",
                                         bufs=2)
                        nc.sync.dma_start(
                            wb[:], w1b_d[blk].rearrange("p (k c) -> p k c",
                                                        c=512))
                    for mi in range(4):
                        m = blk * 4 + mi
                        mc = slice(mi * P, (mi + 1) * P)
                        hb = mpool.tile([P, SP], BF16, tag="hb", bufs=2)
                        for (c0, cw, subs) in CH:
                            ps = psA.tile([P, 512], F32, tag="pA")
                            terms = [(a1, wa), (a2, wa)]
                            if FC1_TERMS == 3:
                                terms.append((a1, wb))
                            nterm = len(terms)
                            nsub = len(subs)
                            for si in range(nsub):
                                s0 = c0 + si * 256
                                sn = subs[si]
                                for tix, (asrc, wsrc) in enumerate(terms):
                                    for kp in range(4):
                                        nc.tensor.matmul(
                                            ps[:, si * 256:si * 256 + sn],
                                            wsrc[:, 2 * kp:2 * kp + 2, mc],
                                            asrc[:, 2 * kp:2 * kp + 2,
                                                 s0:s0 + sn],
                                            start=(si == 0 and tix == 0
                                                   and kp == 0),
                                            stop=(si == nsub - 1 and
                                                  tix == nterm - 1 and
                                                  kp == 3),
                                            perf_mode=DR)
                            nc.scalar.activation(
                                hb[:, c0:c0 + cw], ps[:, :cw], _GELU,
                                bias=b1_sb[:, m:m + 1], scale=1.0 / WSC)
                        nc.gpsimd.tensor_copy(h1[:, m, :], hb[:, :])
                        nc.gpsimd.tensor_tensor(
                            h2[:, m, :], hb[:, :], h1[:, m, :], OP.subtract)

            def gen_fc2(h1, h2, x2, y_sb, mpool, mwpool):
                for blk in range(8):
                    yield
                    wa = mwpool.tile([P, FK, 128], FP8, tag="w2a", bufs=2)
                    nc.sync.dma_start(
                        wa[:], w2a_d[blk].rearrange("p (k c) -> p k c",
                                                    c=128))
                    if FC2_TERMS == 3:
                        wb = mwpool.tile([P, FK, 128], FP8, tag="w2b",
                                         bufs=2)
                        nc.sync.dma_start(
                            wb[:], w2b_d[blk].rearrange("p (k c) -> p k c",
                                                        c=128))
                    for mi in range(1):
                        m = blk
                        mc = slice(0, P)
                        mlp_fm = mpool.tile([P, SP], BF16, tag="mlp_fm",
                                            bufs=2)
                        for (c0, cw, subs) in CH:
                            ps = psA.tile([P, 512], F32, tag="pA")
                            terms = [(h1, wa), (h2, wa)]
                            if FC2_TERMS == 3:
                                terms.append((h1, wb))
                            nterm = len(terms)
                            nsub = len(subs)
                            for si in range(nsub):
                                s0 = c0 + si * 256
                                sn = subs[si]
                                for tix, (hsrc, wsrc) in enumerate(terms):
                                    for kp in range(FK // 2):
                                        nc.tensor.matmul(
                                            ps[:, si * 256:si * 256 + sn],
                                            wsrc[:, 2 * kp:2 * kp + 2, mc],
                                            hsrc[:, 2 * kp:2 * kp + 2,
                                                 s0:s0 + sn],
                                            start=(si == 0 and tix == 0
                                                   and kp == 0),
                                            stop=(si == nsub - 1 and
                                                  tix == nterm - 1 and
                                                  kp == FK // 2 - 1),
                                            perf_mode=DR)
                            nc.vector.tensor_scalar(
                                mlp_fm[:, c0:c0 + cw], ps[:, :cw],
                                1.0 / W2SC, b2_sb[:, m:m + 1],
                                OP.mult, OP.add)
                        # 5 transposes into one bank; single residual op.
                        # (tile0's start zeroes the whole region, so the
                        # garbage rows of tile4 read as zeros.)
                        pst = psA.tile([P, 512], F32, tag="pA")
                        pbf = pst[:].bitcast(BF16).rearrange(
                            "p (t c) -> p t c", c=P)
                        for ti, (t0, pt) in enumerate(TT):
                            nc.tensor.matmul(
                                pbf[:pt, ti, :P], mlp_fm[:, t0:t0 + pt],
                                ident_bf[:], is_transpose=True,
                                start=(ti == 0), stop=(ti == 4))
                        nc.vector.scalar_tensor_tensor(
                            y_sb[:, :, m * P:(m + 1) * P],
                            pbf[:, 0:5, :], 0.0,
                            x2[:, :, m * P:(m + 1) * P],
                            OP.add, OP.add)

            # ======== software-pipelined batch schedule ========
            # A(b)=LN1+QKV, B(b)=heads, C(b)=Oproj+LN2, then fc1/fc2.
            # Emission (=per-engine execution) order:
            #   A0, B0(x)A1, C0, B1(x)FC1(0), C1, FC2(0), FC1(1), FC2(1)
            # so batch-1's ACT/DVE-bound attention is covered by batch-0's
            # MLP matmuls on the PE.
            with tc.tile_pool(name="attn", bufs=1) as apool, \
                 tc.tile_pool(name="mlp", bufs=1) as mpool, \
                 tc.tile_pool(name="wmlp", bufs=1) as mwpool:

                T = [dict() for _ in range(BL)]

                def gen_A(b):
                    t = T[b]
                    xb = rpool.tile([P, 5, D], BF16, tag="xb", bufs=2,
                                    name="xb")
                    t["xb"] = xb
                    load_x(xb, b)
                    stats1 = ln_new_stats()
                    for ti, (t0, pt) in enumerate(TT):
                        ln_tile_stats(stats1, xb, ti, pt)
                    yield
                    xn1 = fmpool.tile([P, KK, SP], FP8, tag="xn1_fm",
                                      name="xn1")
                    ln_finalize(stats1, 0, 2)
                    ln_apply_tiles(stats1, xb, xn1, None, (0, 1))
                    yield
                    ln_finalize(stats1, 2, 5)
                    ln_apply_tiles(stats1, xb, xn1, None, (2, 3, 4))
                    q8 = apool.tile([P, KK, SP], FP8, tag="q8", bufs=2,
                                    name="q8")
                    k8 = apool.tile([P, KK, SP], FP8, tag="k8", bufs=2,
                                    name="k8")
                    v_sb = apool.tile([P, 5, H * VS], FP8, tag="v", bufs=2,
                                      name="v_sb")
                    t["q8"], t["k8"], t["v"] = q8, k8, v_sb
                    yield from gen_qkv(apool, xn1, q8, k8, v_sb)

                def gen_B(b):
                    t = T[b]
                    ctx_fm = apool.tile([P, KK, SP], FP8, tag="ctx",
                                        name="ctx_fm")
                    t["ctx"] = ctx_fm
                    es_cur = apool.tile([P, 5, SP], FP8, tag="es0",
                                        name="es_cur")
                    emit_scores(0, t["q8"], t["k8"], es_cur)
                    for h in range(H):
                        if h + 1 < H:
                            es_nxt = apool.tile([P, 5, SP], FP8,
                                                tag=f"es{(h + 1) % 2}",
                                                name="es_nxt")
                            emit_scores(h + 1, t["q8"], t["k8"], es_nxt)
                        emit_pv(h, es_cur, t["v"], ctx_fm)
                        if h + 1 < H:
                            es_cur = es_nxt
                        yield

                def run_C(b):
                    t = T[b]
                    x2 = rpool.tile([P, 5, D], BF16, tag="x2", bufs=2,
                                    name="x2")
                    t["x2"] = x2
                    stats2 = ln_new_stats()
                    stage_oproj(apool, t["ctx"], t["xb"], x2, stats2)
                    a1 = fmpool.tile([P, KK, SP], FP8, tag="a1", name="a1")
                    a2 = fmpool.tile([P, KK, SP], FP8, tag="a2", name="a2")
                    t["a1"], t["a2"] = a1, a2
                    ln_finalize(stats2, 0, 5)
                    ln_apply_tiles(stats2, x2, a1, a2, (0, 1, 2, 3, 4))

                def gen_mlp(b):
                    t = T[b]
                    h1 = mpool.tile([P, FK, SP], FP8, tag="h1", name="h1")
                    h2 = mpool.tile([P, FK, SP], FP8, tag="h2", name="h2")
                    yield from gen_fc1(t["a1"], t["a2"], h1, h2,
                                       mpool, mwpool)
                    y_sb = ypool.tile([P, 5, D], BF16, tag="y_sb",
                                      name="y_sb")
                    t["y"] = y_sb
                    yield from gen_fc2(h1, h2, t["x2"], y_sb, mpool, mwpool)

                def store_y(b):
                    for ti, (t0, pt) in enumerate(TT):
                        rp = min(pt, S - t0)
                        nc.sync.dma_start(y_d[b, t0:t0 + rp, :],
                                          T[b]["y"][:rp, ti])

                def run_all(g):
                    for _ in g:
                        pass

                def interleave(main, side, ratio):
                    acc = 0.0
                    for _ in main:
                        acc += ratio
                        while acc >= 1.0:
                            next(side, None)
                            acc -= 1.0
                    run_all(side)

                run_all(gen_A(0))
                interleave(gen_B(0), gen_A(1), 10.0 / H)
                run_C(0)
                mlp0 = gen_mlp(0)
                # fc1 is 8 of mlp0's 12 units; pump them under B1's heads
                interleave(gen_B(1), mlp0, 8.0 / H)
                run_C(1)
                run_all(mlp0)
                store_y(0)
                run_all(gen_mlp(1))
                store_y(1)

    nc.compile()
    return nc


def _get_nc():
    global _NC_CACHE
    if _NC_CACHE is None:
        _NC_CACHE = _build()
    return _NC_CACHE


def kernel(**inputs):
    nc = _get_nc()
    shared = prepare_shared(inputs)
    x = np.asarray(inputs["x"], np.float32).astype(BFNP)
    in_maps = []
    for i in range(NCORES):
        m = dict(shared)
        m["x"] = np.ascontiguousarray(x[i * BL:(i + 1) * BL])
        in_maps.append(m)
    res = bass_utils.run_bass_kernel_spmd(nc, in_maps,
                                          core_ids=list(range(NCORES)))
    y = np.concatenate([np.asarray(res.results[i]["y"])
                        for i in range(NCORES)], axis=0)
    return y.astype(np.float32)
